# revision 5
# baseline (speedup 1.0000x reference)
# Trainium2 Bass kernel for nn_AutoregressiveLSTM (2-layer LSTM encode over
# T=512 steps + 64 autoregressive decode steps with BN+FC+feedback).
#
# Strategy (per core, batch-sharded 8 ways, B_loc=512):
#   - States/gates live as [H or 4H on partitions, B on free] tiles.
#   - Weights are stationary lhsT operands (bf16); h is the moving operand.
#   - Gate PSUM bank layout: [i0, f0, o0, i1, f1, o1, g0, g1] so one
#     bank-spanning Sigmoid covers i/f/o of a layer and one Tanh covers g.
#   - L0 input contribution + bias: K=9 row-tiled matmuls (4 concurrent 32-row
#     PE groups) against a host-prepped x tile that carries a ones-row per
#     group (bias rides the matmul). L1 bias: K=1 row-tiled matmuls vs the
#     same ones rows.
#   - Encode runs L1 lagged one slot behind L0 so both layers' engine work
#     overlaps inside a slot.
#   - Decode folds BN into FC1 and folds (feedback linear ∘ FC2 ∘ Wih0) into
#     a single K=65 matmul from the ReLU activations, removing the feedback
#     path from the serial chain. FC2 itself only feeds the output DMA.
#   - bf16 everywhere on-chip except PSUM accumulation (fp32) and the final
#     y copy (fp32): the LSTM recurrence is contractive, bf16 error stays
#     ~0.5% rms (measured) instead of accumulating.
import os
import sys

import numpy as np

if "/opt/trn_rl_repo" not in sys.path:
    sys.path.insert(0, "/opt/trn_rl_repo")

import ml_dtypes  # noqa: E402
import concourse.tile as tile  # noqa: E402
from concourse import bacc, mybir  # noqa: E402
from concourse.bass_utils import run_bass_kernel_spmd  # noqa: E402

F32 = mybir.dt.float32
BF16 = mybir.dt.bfloat16
AF = mybir.ActivationFunctionType

N_CORES = 8
H = 128
IN = 8
OUT = 4
FC_H = 64
BN_EPS = 1e-5
B_LOC = 512

# gate row ranges in the PyTorch weight layout (i, f, g, o)
GATE_ROWS = {
    "i": slice(0, H),
    "f": slice(H, 2 * H),
    "g": slice(2 * H, 3 * H),
    "o": slice(3 * H, 4 * H),
}
# chunk k (PE row-group k, lhsT column block k) holds gate CHUNKS[k]
CHUNKS = ["i", "f", "o", "g"]
L0_BANK = {"i": 0, "f": 1, "o": 2, "g": 6}
L1_BANK = {"i": 3, "f": 4, "o": 5, "g": 6}
N_WARM_DUMMIES = int(os.environ.get("N_WARM_DUMMIES", "5"))

_CACHE = {}


def _build_program(T, PRED):
    nc = bacc.Bacc(
        "TRN2",
        target_bir_lowering=False,
        debug=False,
        enable_asserts=False,
        num_devices=N_CORES,
    )

    d_x = nc.dram_tensor("x_enc", (T, 128, B_LOC), BF16, kind="ExternalInput")
    d_w0x4 = nc.dram_tensor("w0x4", (128, 128), BF16, kind="ExternalInput")
    d_b14 = nc.dram_tensor("b14", (128, 128), BF16, kind="ExternalInput")
    d_w0h = nc.dram_tensor("w0h", (H, 512), BF16, kind="ExternalInput")
    d_w1i = nc.dram_tensor("w1i", (H, 512), BF16, kind="ExternalInput")
    d_w1h = nc.dram_tensor("w1h", (H, 512), BF16, kind="ExternalInput")
    d_wfc1 = nc.dram_tensor("wfc1", (H, FC_H), BF16, kind="ExternalInput")
    d_b1p = nc.dram_tensor("b1p", (FC_H, 1), F32, kind="ExternalInput")
    d_wfc2 = nc.dram_tensor("wfc2", (FC_H, OUT), BF16, kind="ExternalInput")
    d_b2 = nc.dram_tensor("b2", (OUT, 1), F32, kind="ExternalInput")
    d_w0r = nc.dram_tensor("w0r", (FC_H + 1, 512), BF16, kind="ExternalInput")
    d_y = nc.dram_tensor("y", (PRED, OUT, B_LOC), F32, kind="ExternalOutput")

    from contextlib import ExitStack

    with tile.TileContext(nc) as tc, ExitStack() as ctx:
        wpool = ctx.enter_context(tc.tile_pool(name="w", bufs=1))
        spool = ctx.enter_context(tc.tile_pool(name="s", bufs=1))
        gpool = ctx.enter_context(tc.tile_pool(name="g", bufs=2))
        xpool = ctx.enter_context(tc.tile_pool(name="x", bufs=6))
        opool = ctx.enter_context(tc.tile_pool(name="o", bufs=3))
        ppool = ctx.enter_context(tc.tile_pool(name="p", bufs=1, space="PSUM"))

        t_w0x4 = wpool.tile([128, 128], BF16)
        nc.sync.dma_start(t_w0x4[:], d_w0x4.ap())
        t_b14 = wpool.tile([128, 128], BF16)
        nc.sync.dma_start(t_b14[:], d_b14.ap())
        t_w0h = wpool.tile([H, 512], BF16)
        nc.sync.dma_start(t_w0h[:], d_w0h.ap())
        t_w1i = wpool.tile([H, 512], BF16)
        nc.sync.dma_start(t_w1i[:], d_w1i.ap())
        t_w1h = wpool.tile([H, 512], BF16)
        nc.sync.dma_start(t_w1h[:], d_w1h.ap())
        t_wfc1 = wpool.tile([H, FC_H], BF16)
        nc.sync.dma_start(t_wfc1[:], d_wfc1.ap())
        t_b1p = wpool.tile([FC_H, 1], F32)
        nc.sync.dma_start(t_b1p[:], d_b1p.ap())
        t_wfc2 = wpool.tile([FC_H, OUT], BF16)
        nc.sync.dma_start(t_wfc2[:], d_wfc2.ap())
        t_b2 = wpool.tile([OUT, 1], F32)
        nc.sync.dma_start(t_b2[:], d_b2.ap())
        t_w0r = wpool.tile([FC_H + 1, 512], BF16)
        nc.sync.dma_start(t_w0r[:], d_w0r.ap())

        h0 = spool.tile([H, B_LOC], BF16)
        c0 = spool.tile([H, B_LOC], BF16)
        h1 = spool.tile([H, B_LOC], BF16)
        c1 = spool.tile([H, B_LOC], BF16)
        for t_ in (h0, c0, h1, c1):
            nc.vector.memset(t_[:], 0.0)

        # r_aug: ReLU activations (rows 0..63) + ones row 64 for decode folds
        r_aug = spool.tile([FC_H + 1, B_LOC], BF16)
        nc.vector.memset(r_aug[64:65, :], 1.0)

        psum = ppool.tile([128, 8 * 512], F32)

        def bank(b):
            return psum[:, 512 * b : 512 * (b + 1)]

        def l0_cell(xt):
            """L0 gates already prefilled with x-part+bias into L0 banks by
            the caller; this adds the recurrent part and runs the pointwise
            chain. Updates h0/c0."""
            for gname in ("g", "i", "f", "o"):
                k = CHUNKS.index(gname)
                nc.tensor.matmul(
                    bank(L0_BANK[gname]),
                    t_w0h[:, 128 * k : 128 * (k + 1)],
                    h0[:],
                    start=False,
                    stop=True,
                )
            g0t = gpool.tile([H, 512], BF16, name="g0t")
            nc.scalar.activation(g0t[:], bank(6), AF.Tanh)
            ifo0 = gpool.tile([H, 1536], BF16, name="ifo0")
            nc.scalar.activation(ifo0[:], psum[:, 0:1536], AF.Sigmoid)
            v0 = gpool.tile([H, 512], BF16, name="v0")
            nc.vector.tensor_mul(v0[:], ifo0[:, 0:512], g0t[:])
            u0 = gpool.tile([H, 512], BF16, name="u0")
            nc.vector.tensor_mul(u0[:], ifo0[:, 512:1024], c0[:])
            nc.vector.tensor_add(c0[:], u0[:], v0[:])
            tc0 = gpool.tile([H, 512], BF16, name="tc0")
            nc.scalar.activation(tc0[:], c0[:], AF.Tanh)
            nc.vector.tensor_mul(h0[:], ifo0[:, 1024:1536], tc0[:])

        def l1_cell(h0_src, xt_ones):
            """L1 gates: i/f/o bias prefilled into L1 banks; g bias emitted
            here (bank 6 becomes free once tanh_g0 has read g0). Adds input
            and recurrent parts, runs pointwise chain. Updates h1/c1."""
            wave_a_l1(xt_ones, gates=("g",))
            for gname in ("g", "i", "f", "o"):
                k = CHUNKS.index(gname)
                nc.tensor.matmul(
                    bank(L1_BANK[gname]),
                    t_w1i[:, 128 * k : 128 * (k + 1)],
                    h0_src[:],
                    start=False,
                    stop=False,
                )
                nc.tensor.matmul(
                    bank(L1_BANK[gname]),
                    t_w1h[:, 128 * k : 128 * (k + 1)],
                    h1[:],
                    start=False,
                    stop=True,
                )
            g1t = gpool.tile([H, 512], BF16, name="g1t")
            nc.scalar.activation(g1t[:], bank(6), AF.Tanh)
            ifo1 = gpool.tile([H, 1536], BF16, name="ifo1")
            nc.scalar.activation(ifo1[:], psum[:, 1536:3072], AF.Sigmoid)
            v1 = gpool.tile([H, 512], BF16, name="v1")
            nc.vector.tensor_mul(v1[:], ifo1[:, 0:512], g1t[:])
            u1 = gpool.tile([H, 512], BF16, name="u1")
            nc.vector.tensor_mul(u1[:], ifo1[:, 512:1024], c1[:])
            nc.vector.tensor_add(c1[:], u1[:], v1[:])
            tc1 = gpool.tile([H, 512], BF16, name="tc1")
            nc.scalar.activation(tc1[:], c1[:], AF.Tanh)
            nc.vector.tensor_mul(h1[:], ifo1[:, 1024:1536], tc1[:])

        def wave_a_l0(xt):
            # row-tiled K=9 x-part (+ bias via ones row) into L0 banks
            for k, gname in enumerate(CHUNKS):
                nc.tensor.matmul(
                    bank(L0_BANK[gname]),
                    t_w0x4[32 * k : 32 * k + 9, :],
                    xt[32 * k : 32 * k + 9, :],
                    start=True,
                    stop=False,
                    tile_position=(32 * k, 0),
                )

        def wave_a_l1(xt, gates=("i", "f", "o")):
            # row-tiled K=1 L1-bias (vs ones rows of xt) into L1 banks.
            # The g-gate bias shares bank 6 with g0, so it is emitted
            # separately after tanh_g0 has consumed g0.
            for gname in gates:
                k = CHUNKS.index(gname)
                nc.tensor.matmul(
                    bank(L1_BANK[gname]),
                    t_b14[32 * k : 32 * k + 1, :],
                    xt[32 * k : 32 * k + 1, :],
                    start=True,
                    stop=False,
                    tile_position=(32 * k, 0),
                )

        def warm_dummies(n):
            # keep-warm matmuls into the sacrificial bank 7: HAM re-throttles
            # the PE clock to 1.2 GHz after idle windows; these fill the gaps.
            for _ in range(n):
                nc.tensor.matmul(
                    bank(7), t_b14[0:1, :], t_w0h[0:1, :], start=True, stop=True
                )

        # ---------------- encode: slots 0..T-1 (L1 lagged by 1) -------------
        x_tiles = [None, None]  # remember last x tile for decode step 0
        for t in range(T):
            xt = xpool.tile([128, B_LOC], BF16, name="xt")
            nc.sync.dma_start(xt[:], d_x.ap()[t])
            x_tiles[t % 2] = xt

            wave_a_l0(xt)
            if t > 0:
                wave_a_l1(xt)  # i/f/o biases for lagged L1 cell t-1
            for gname in ("g", "i", "f", "o"):
                k = CHUNKS.index(gname)
                nc.tensor.matmul(
                    bank(L0_BANK[gname]),
                    t_w0h[:, 128 * k : 128 * (k + 1)],
                    h0[:],
                    start=False,
                    stop=True,
                )
            # L0 activations (free banks 6 and 0-2 for L1 / next slot)
            g0t = gpool.tile([H, 512], BF16, name="g0t")
            nc.scalar.activation(g0t[:], bank(6), AF.Tanh)
            ifo0 = gpool.tile([H, 1536], BF16, name="ifo0")
            nc.scalar.activation(ifo0[:], psum[:, 0:1536], AF.Sigmoid)
            if t > 0:
                # lagged L1 cell t-1: g-bias takes bank 6 after tanh_g0;
                # ih1/hh1 read h0_{t-1} / h1_{t-2} (emitted before the h0/h1
                # updates below, so Tile sequences them on the old values)
                wave_a_l1(xt, gates=("g",))
                for gname in ("g", "i", "f", "o"):
                    k = CHUNKS.index(gname)
                    nc.tensor.matmul(
                        bank(L1_BANK[gname]),
                        t_w1i[:, 128 * k : 128 * (k + 1)],
                        h0[:],
                        start=False,
                        stop=False,
                    )
                    nc.tensor.matmul(
                        bank(L1_BANK[gname]),
                        t_w1h[:, 128 * k : 128 * (k + 1)],
                        h1[:],
                        start=False,
                        stop=True,
                    )
            # L0 pointwise chain (updates h0, c0)
            v0 = gpool.tile([H, 512], BF16, name="v0")
            nc.vector.tensor_mul(v0[:], ifo0[:, 0:512], g0t[:])
            u0 = gpool.tile([H, 512], BF16, name="u0")
            nc.vector.tensor_mul(u0[:], ifo0[:, 512:1024], c0[:])
            nc.vector.tensor_add(c0[:], u0[:], v0[:])
            tc0 = gpool.tile([H, 512], BF16, name="tc0")
            nc.scalar.activation(tc0[:], c0[:], AF.Tanh)
            nc.vector.tensor_mul(h0[:], ifo0[:, 1024:1536], tc0[:])
            if t > 0:
                # L1 pointwise chain (updates h1, c1) for lagged cell t-1
                g1t = gpool.tile([H, 512], BF16, name="g1t")
                nc.scalar.activation(g1t[:], bank(6), AF.Tanh)
                ifo1 = gpool.tile([H, 1536], BF16, name="ifo1")
                nc.scalar.activation(ifo1[:], psum[:, 1536:3072], AF.Sigmoid)
                v1 = gpool.tile([H, 512], BF16, name="v1")
                nc.vector.tensor_mul(v1[:], ifo1[:, 0:512], g1t[:])
                u1 = gpool.tile([H, 512], BF16, name="u1")
                nc.vector.tensor_mul(u1[:], ifo1[:, 512:1024], c1[:])
                nc.vector.tensor_add(c1[:], u1[:], v1[:])
                tc1 = gpool.tile([H, 512], BF16, name="tc1")
                nc.scalar.activation(tc1[:], c1[:], AF.Tanh)
                nc.vector.tensor_mul(h1[:], ifo1[:, 1024:1536], tc1[:])
            if N_WARM_DUMMIES and t > 0:
                warm_dummies(N_WARM_DUMMIES)

        # encode epilogue: L1 cell T-1 (uses final h0 and the last x tile's
        # ones rows for bias)
        xlast = x_tiles[(T - 1) % 2]
        wave_a_l1(xlast)
        l1_cell(h0, xlast)

        # ---------------- decode: steps 0..PRED-1 ----------------------------
        for s in range(PRED):
            if s == 0:
                # x-part from the last input timestep (same tile as encode
                # slot T-1), row-tiled K=9 + bias
                wave_a_l0(xlast)
            else:
                # folded input path: gates_x = W0r @ r_aug (K=65, incl. bias)
                for k, gname in enumerate(CHUNKS):
                    nc.tensor.matmul(
                        bank(L0_BANK[gname]),
                        t_w0r[:, 128 * k : 128 * (k + 1)],
                        r_aug[:],
                        start=True,
                        stop=False,
                    )
            wave_a_l1(xlast)  # K=1 bias rides xlast's ones rows every step
            l0_cell(None)
            l1_cell(h0, xlast)

            # FC1 + ReLU (BN folded into weights; bias via activation bias)
            nc.tensor.matmul(bank(6)[0:FC_H, :], t_wfc1[:], h1[:], start=True, stop=True)
            nc.scalar.activation(
                r_aug[0:FC_H, :], bank(6)[0:FC_H, :], AF.Relu, bias=t_b1p[:]
            )
            # FC2 (output only) + bias via copy activation
            nc.tensor.matmul(
                bank(7)[0:OUT, :], t_wfc2[:], r_aug[0:FC_H, :], start=True, stop=True
            )
            y_sb = opool.tile([OUT, B_LOC], F32, name="y_sb")
            nc.scalar.activation(y_sb[:], bank(7)[0:OUT, :], AF.Identity, bias=t_b2[:])
            nc.sync.dma_start(d_y.ap()[s], y_sb[:])

    nc.compile()
    return nc


def _host_prep(inputs):
    """Build per-core input maps from the full problem inputs."""
    bf = ml_dtypes.bfloat16
    x = np.asarray(inputs["x"], np.float32)
    Bfull, T, _ = x.shape
    W0x = np.asarray(inputs["Wih0"], np.float32)
    W0h = np.asarray(inputs["Whh0"], np.float32)
    b0 = np.asarray(inputs["bih0"], np.float32) + np.asarray(inputs["bhh0"], np.float32)
    W1i = np.asarray(inputs["Wih1"], np.float32)
    W1h = np.asarray(inputs["Whh1"], np.float32)
    b1 = np.asarray(inputs["bih1"], np.float32) + np.asarray(inputs["bhh1"], np.float32)
    gamma = np.asarray(inputs["bn_gamma"], np.float32)
    beta = np.asarray(inputs["bn_beta"], np.float32)
    mean = np.asarray(inputs["bn_mean"], np.float32)
    var = np.asarray(inputs["bn_var"], np.float32)
    W1 = np.asarray(inputs["W1"], np.float32)
    b1f = np.asarray(inputs["b1"], np.float32)
    W2 = np.asarray(inputs["W2"], np.float32)
    b2 = np.asarray(inputs["b2"], np.float32)
    Wf = np.asarray(inputs["Wf"], np.float32)
    bfb = np.asarray(inputs["bf"], np.float32)

    # BN fold into FC1
    scale = gamma / np.sqrt(var + BN_EPS)
    W1p = W1 * scale[None, :]
    b1p = b1f + W1 @ (beta - mean * scale)

    # feedback folds
    Wf2 = Wf @ W2  # [8, 64]
    bff = Wf @ b2 + bfb  # [8]
    W0r = W0x @ Wf2  # [4H, 64]
    b0r = W0x @ bff + b0  # [4H]

    # shared weight tiles
    w0x4 = np.zeros((128, 128), np.float32)
    b14 = np.zeros((128, 128), np.float32)
    for k, gname in enumerate(CHUNKS):
        rows = GATE_ROWS[gname]
        w0x4[32 * k, :] = b0[rows]
        w0x4[32 * k + 1 : 32 * k + 9, :] = W0x[rows].T
        b14[32 * k, :] = b1[rows]

    def lhsT_of(W):
        out = np.zeros((H, 512), np.float32)
        for k, gname in enumerate(CHUNKS):
            out[:, 128 * k : 128 * (k + 1)] = W[GATE_ROWS[gname]].T
        return out

    w0r = np.zeros((FC_H + 1, 512), np.float32)
    for k, gname in enumerate(CHUNKS):
        rows = GATE_ROWS[gname]
        w0r[0:FC_H, 128 * k : 128 * (k + 1)] = W0r[rows].T
        w0r[FC_H, 128 * k : 128 * (k + 1)] = b0r[rows]

    shared = {
        "w0x4": w0x4.astype(bf),
        "b14": b14.astype(bf),
        "w0h": lhsT_of(W0h).astype(bf),
        "w1i": lhsT_of(W1i).astype(bf),
        "w1h": lhsT_of(W1h).astype(bf),
        "wfc1": W1p.T.astype(bf),
        "b1p": b1p.reshape(FC_H, 1).astype(np.float32),
        "wfc2": W2.T.astype(bf),
        "b2": b2.reshape(OUT, 1).astype(np.float32),
        "w0r": w0r.astype(bf),
    }

    in_maps = []
    bper = Bfull // N_CORES
    for ci in range(N_CORES):
        xc = x[ci * bper : (ci + 1) * bper]  # [B_loc, T, 8]
        xt = np.ascontiguousarray(xc.transpose(1, 2, 0))  # [T, 8, B_loc]
        x_enc = np.zeros((T, 128, bper), np.float32)
        for k in range(4):
            x_enc[:, 32 * k, :] = 1.0
            x_enc[:, 32 * k + 1 : 32 * k + 9, :] = xt
        m = dict(shared)
        m["x_enc"] = x_enc.astype(bf)
        in_maps.append(m)
    return in_maps


def kernel(**inputs) -> np.ndarray:
    x = np.asarray(inputs["x"])
    Bfull, T, _ = x.shape
    PRED = int(inputs["prediction_steps"])
    key = (T, PRED)
    if key not in _CACHE:
        _CACHE[key] = _build_program(T, PRED)
    nc = _CACHE[key]

    in_maps = _host_prep(inputs)
    trace = os.environ.get("KERNEL_TRACE", "0") == "1"
    if trace:
        try:
            from harness import install_ntff_hook

            install_ntff_hook()
        except Exception:
            trace = False
    res = run_bass_kernel_spmd(
        nc,
        in_maps,
        core_ids=list(range(N_CORES)),
        trace=trace,
        tmpdir=os.environ.get("KERNEL_TRACE_DIR") or None,
    )
    if trace and res.exec_time_ns is not None:
        print(f"HW exec time: {res.exec_time_ns} ns")

    bper = Bfull // N_CORES
    out = np.empty((Bfull, PRED, OUT), np.float32)
    for ci in range(N_CORES):
        y = res.results[ci]["y"]  # [PRED, OUT, B_loc]
        out[ci * bper : (ci + 1) * bper] = y.transpose(2, 0, 1)
    return out


# revision 6
# speedup vs baseline: 1.0093x; 1.0093x over previous
# Trainium2 Bass kernel for nn_AutoregressiveLSTM (2-layer LSTM encode over
# T=512 steps + 64 autoregressive decode steps with BN+FC+feedback).
#
# Strategy (per core, batch-sharded 8 ways, B_loc=512):
#   - States/gates live as [H or 4H on partitions, B on free] tiles.
#   - Weights are stationary lhsT operands (bf16); h is the moving operand.
#   - Gate PSUM bank layout: [i0, f0, o0, i1, f1, o1, g0, g1] so one
#     bank-spanning Sigmoid covers i/f/o of a layer and one Tanh covers g.
#   - L0 input contribution + bias: K=9 row-tiled matmuls (4 concurrent 32-row
#     PE groups) against a host-prepped x tile that carries a ones-row per
#     group (bias rides the matmul). L1 bias: K=1 row-tiled matmuls vs the
#     same ones rows.
#   - Encode runs L1 lagged one slot behind L0 so both layers' engine work
#     overlaps inside a slot.
#   - Decode folds BN into FC1 and folds (feedback linear ∘ FC2 ∘ Wih0) into
#     a single K=65 matmul from the ReLU activations, removing the feedback
#     path from the serial chain. FC2 itself only feeds the output DMA.
#   - bf16 everywhere on-chip except PSUM accumulation (fp32) and the final
#     y copy (fp32): the LSTM recurrence is contractive, bf16 error stays
#     ~0.5% rms (measured) instead of accumulating.
import os
import sys

import numpy as np

if "/opt/trn_rl_repo" not in sys.path:
    sys.path.insert(0, "/opt/trn_rl_repo")

import ml_dtypes  # noqa: E402
import concourse.tile as tile  # noqa: E402
from concourse import bacc, mybir  # noqa: E402
from concourse.bass_utils import run_bass_kernel_spmd  # noqa: E402

F32 = mybir.dt.float32
BF16 = mybir.dt.bfloat16
AF = mybir.ActivationFunctionType

N_CORES = 8
H = 128
IN = 8
OUT = 4
FC_H = 64
BN_EPS = 1e-5
B_LOC = 512

# gate row ranges in the PyTorch weight layout (i, f, g, o)
GATE_ROWS = {
    "i": slice(0, H),
    "f": slice(H, 2 * H),
    "g": slice(2 * H, 3 * H),
    "o": slice(3 * H, 4 * H),
}
# chunk k (PE row-group k, lhsT column block k) holds gate CHUNKS[k]
CHUNKS = ["i", "f", "o", "g"]
L0_BANK = {"i": 0, "f": 1, "o": 2, "g": 6}
L1_BANK = {"i": 3, "f": 4, "o": 5, "g": 6}
N_WARM_DUMMIES = int(os.environ.get("N_WARM_DUMMIES", "5"))

_CACHE = {}


def _build_program(T, PRED):
    nc = bacc.Bacc(
        "TRN2",
        target_bir_lowering=False,
        debug=False,
        enable_asserts=False,
        num_devices=N_CORES,
    )

    d_x = nc.dram_tensor("x_enc", (T, 128, B_LOC), BF16, kind="ExternalInput")
    d_w0x4 = nc.dram_tensor("w0x4", (128, 128), BF16, kind="ExternalInput")
    d_b14 = nc.dram_tensor("b14", (128, 128), BF16, kind="ExternalInput")
    d_w0h = nc.dram_tensor("w0h", (H, 512), BF16, kind="ExternalInput")
    d_w1i = nc.dram_tensor("w1i", (H, 512), BF16, kind="ExternalInput")
    d_w1h = nc.dram_tensor("w1h", (H, 512), BF16, kind="ExternalInput")
    d_wfc1 = nc.dram_tensor("wfc1", (H, FC_H), BF16, kind="ExternalInput")
    d_b1p = nc.dram_tensor("b1p", (FC_H, 1), F32, kind="ExternalInput")
    d_wfc2 = nc.dram_tensor("wfc2", (FC_H, OUT), BF16, kind="ExternalInput")
    d_b2 = nc.dram_tensor("b2", (OUT, 1), F32, kind="ExternalInput")
    d_w0r = nc.dram_tensor("w0r", (FC_H + 1, 512), BF16, kind="ExternalInput")
    d_b1g = nc.dram_tensor("b1g", (H, 1), F32, kind="ExternalInput")
    d_y = nc.dram_tensor("y", (PRED, OUT, B_LOC), F32, kind="ExternalOutput")

    from contextlib import ExitStack

    with tile.TileContext(nc) as tc, ExitStack() as ctx:
        wpool = ctx.enter_context(tc.tile_pool(name="w", bufs=1))
        spool = ctx.enter_context(tc.tile_pool(name="s", bufs=1))
        gpool = ctx.enter_context(tc.tile_pool(name="g", bufs=2))
        xpool = ctx.enter_context(tc.tile_pool(name="x", bufs=6))
        opool = ctx.enter_context(tc.tile_pool(name="o", bufs=3))
        ppool = ctx.enter_context(tc.tile_pool(name="p", bufs=1, space="PSUM"))

        t_w0x4 = wpool.tile([128, 128], BF16)
        nc.sync.dma_start(t_w0x4[:], d_w0x4.ap())
        t_b14 = wpool.tile([128, 128], BF16)
        nc.sync.dma_start(t_b14[:], d_b14.ap())
        t_w0h = wpool.tile([H, 512], BF16)
        nc.sync.dma_start(t_w0h[:], d_w0h.ap())
        t_w1i = wpool.tile([H, 512], BF16)
        nc.sync.dma_start(t_w1i[:], d_w1i.ap())
        t_w1h = wpool.tile([H, 512], BF16)
        nc.sync.dma_start(t_w1h[:], d_w1h.ap())
        t_wfc1 = wpool.tile([H, FC_H], BF16)
        nc.sync.dma_start(t_wfc1[:], d_wfc1.ap())
        t_b1p = wpool.tile([FC_H, 1], F32)
        nc.sync.dma_start(t_b1p[:], d_b1p.ap())
        t_wfc2 = wpool.tile([FC_H, OUT], BF16)
        nc.sync.dma_start(t_wfc2[:], d_wfc2.ap())
        t_b2 = wpool.tile([OUT, 1], F32)
        nc.sync.dma_start(t_b2[:], d_b2.ap())
        t_w0r = wpool.tile([FC_H + 1, 512], BF16)
        nc.sync.dma_start(t_w0r[:], d_w0r.ap())
        t_b1g = wpool.tile([H, 1], F32)
        nc.sync.dma_start(t_b1g[:], d_b1g.ap())

        h0 = spool.tile([H, B_LOC], BF16)
        c0 = spool.tile([H, B_LOC], BF16)
        h1 = spool.tile([H, B_LOC], BF16)
        c1 = spool.tile([H, B_LOC], BF16)
        for t_ in (h0, c0, h1, c1):
            nc.vector.memset(t_[:], 0.0)

        # r_aug: ReLU activations (rows 0..63) + ones row 64 for decode folds
        r_aug = spool.tile([FC_H + 1, B_LOC], BF16)
        nc.vector.memset(r_aug[64:65, :], 1.0)

        psum = ppool.tile([128, 8 * 512], F32)

        def bank(b):
            return psum[:, 512 * b : 512 * (b + 1)]

        def l0_cell(xt):
            """L0 gates already prefilled with x-part+bias into L0 banks by
            the caller; this adds the recurrent part and runs the pointwise
            chain. Updates h0/c0."""
            for gname in ("g", "i", "f", "o"):
                k = CHUNKS.index(gname)
                nc.tensor.matmul(
                    bank(L0_BANK[gname]),
                    t_w0h[:, 128 * k : 128 * (k + 1)],
                    h0[:],
                    start=False,
                    stop=True,
                )
            g0t = gpool.tile([H, 512], BF16, name="g0t")
            nc.scalar.activation(g0t[:], bank(6), AF.Tanh)
            ifo0 = gpool.tile([H, 1536], BF16, name="ifo0")
            nc.scalar.activation(ifo0[:], psum[:, 0:1536], AF.Sigmoid)
            v0 = gpool.tile([H, 512], BF16, name="v0")
            nc.vector.tensor_mul(v0[:], ifo0[:, 0:512], g0t[:])
            u0 = gpool.tile([H, 512], BF16, name="u0")
            nc.vector.tensor_mul(u0[:], ifo0[:, 512:1024], c0[:])
            nc.vector.tensor_add(c0[:], u0[:], v0[:])
            tc0 = gpool.tile([H, 512], BF16, name="tc0")
            nc.scalar.activation(tc0[:], c0[:], AF.Tanh)
            nc.vector.tensor_mul(h0[:], ifo0[:, 1024:1536], tc0[:])

        def l1_cell(h0_src, xt_ones):
            """L1 gates: i/f/o bias prefilled into L1 banks; g bias emitted
            here (bank 6 becomes free once tanh_g0 has read g0). Adds input
            and recurrent parts, runs pointwise chain. Updates h1/c1."""
            for gname in ("g", "i", "f", "o"):
                k = CHUNKS.index(gname)
                nc.tensor.matmul(
                    bank(L1_BANK[gname]),
                    t_w1i[:, 128 * k : 128 * (k + 1)],
                    h0_src[:],
                    start=(gname == "g"),
                    stop=False,
                )
                nc.tensor.matmul(
                    bank(L1_BANK[gname]),
                    t_w1h[:, 128 * k : 128 * (k + 1)],
                    h1[:],
                    start=False,
                    stop=True,
                )
            g1t = gpool.tile([H, 512], BF16, name="g1t")
            nc.scalar.activation(g1t[:], bank(6), AF.Tanh, bias=t_b1g[:])
            ifo1 = gpool.tile([H, 1536], BF16, name="ifo1")
            nc.scalar.activation(ifo1[:], psum[:, 1536:3072], AF.Sigmoid)
            v1 = gpool.tile([H, 512], BF16, name="v1")
            nc.vector.tensor_mul(v1[:], ifo1[:, 0:512], g1t[:])
            u1 = gpool.tile([H, 512], BF16, name="u1")
            nc.vector.tensor_mul(u1[:], ifo1[:, 512:1024], c1[:])
            nc.vector.tensor_add(c1[:], u1[:], v1[:])
            tc1 = gpool.tile([H, 512], BF16, name="tc1")
            nc.scalar.activation(tc1[:], c1[:], AF.Tanh)
            nc.vector.tensor_mul(h1[:], ifo1[:, 1024:1536], tc1[:])

        def wave_a_l0(xt):
            # row-tiled K=9 x-part (+ bias via ones row) into L0 banks
            for k, gname in enumerate(CHUNKS):
                nc.tensor.matmul(
                    bank(L0_BANK[gname]),
                    t_w0x4[32 * k : 32 * k + 9, :],
                    xt[32 * k : 32 * k + 9, :],
                    start=True,
                    stop=False,
                    tile_position=(32 * k, 0),
                )

        def wave_a_l1(xt, gates=("i", "f", "o")):
            # row-tiled K=1 L1-bias (vs ones rows of xt) into L1 banks.
            # The g-gate bias shares bank 6 with g0, so it is emitted
            # separately after tanh_g0 has consumed g0.
            for gname in gates:
                k = CHUNKS.index(gname)
                nc.tensor.matmul(
                    bank(L1_BANK[gname]),
                    t_b14[32 * k : 32 * k + 1, :],
                    xt[32 * k : 32 * k + 1, :],
                    start=True,
                    stop=False,
                    tile_position=(32 * k, 0),
                )

        def warm_dummies(n):
            # keep-warm matmuls into the sacrificial bank 7: HAM re-throttles
            # the PE clock to 1.2 GHz after idle windows; these fill the gaps.
            for _ in range(n):
                nc.tensor.matmul(
                    bank(7), t_b14[0:1, :], t_w0h[0:1, :], start=True, stop=True
                )

        # ---------------- encode: slots 0..T-1 (L1 lagged by 1) -------------
        x_tiles = [None, None]  # remember last x tile for decode step 0
        for t in range(T):
            xt = xpool.tile([128, B_LOC], BF16, name="xt")
            nc.sync.dma_start(xt[:], d_x.ap()[t])
            x_tiles[t % 2] = xt

            wave_a_l0(xt)
            if t > 0:
                wave_a_l1(xt)  # i/f/o biases for lagged L1 cell t-1
            for gname in ("g", "i", "f", "o"):
                k = CHUNKS.index(gname)
                nc.tensor.matmul(
                    bank(L0_BANK[gname]),
                    t_w0h[:, 128 * k : 128 * (k + 1)],
                    h0[:],
                    start=False,
                    stop=True,
                )
            # L0 activations (free banks 6 and 0-2 for L1 / next slot)
            g0t = gpool.tile([H, 512], BF16, name="g0t")
            nc.scalar.activation(g0t[:], bank(6), AF.Tanh)
            ifo0 = gpool.tile([H, 1536], BF16, name="ifo0")
            nc.scalar.activation(ifo0[:], psum[:, 0:1536], AF.Sigmoid)
            if t > 0:
                # lagged L1 cell t-1: bank 6 is reused for g1 after tanh_g0
                # (ih1_g restarts the accumulation); the g-bias is applied via
                # tanh_g1's per-partition bias operand instead of a matmul.
                # ih1/hh1 read h0_{t-1} / h1_{t-2} (emitted before the h0/h1
                # updates below, so Tile sequences them on the old values)
                for gname in ("g", "i", "f", "o"):
                    k = CHUNKS.index(gname)
                    nc.tensor.matmul(
                        bank(L1_BANK[gname]),
                        t_w1i[:, 128 * k : 128 * (k + 1)],
                        h0[:],
                        start=(gname == "g"),
                        stop=False,
                    )
                    nc.tensor.matmul(
                        bank(L1_BANK[gname]),
                        t_w1h[:, 128 * k : 128 * (k + 1)],
                        h1[:],
                        start=False,
                        stop=True,
                    )
            # L0 pointwise chain (updates h0, c0)
            v0 = gpool.tile([H, 512], BF16, name="v0")
            nc.vector.tensor_mul(v0[:], ifo0[:, 0:512], g0t[:])
            u0 = gpool.tile([H, 512], BF16, name="u0")
            nc.vector.tensor_mul(u0[:], ifo0[:, 512:1024], c0[:])
            nc.vector.tensor_add(c0[:], u0[:], v0[:])
            tc0 = gpool.tile([H, 512], BF16, name="tc0")
            nc.scalar.activation(tc0[:], c0[:], AF.Tanh)
            nc.vector.tensor_mul(h0[:], ifo0[:, 1024:1536], tc0[:])
            if t > 0:
                # L1 pointwise chain (updates h1, c1) for lagged cell t-1
                g1t = gpool.tile([H, 512], BF16, name="g1t")
                nc.scalar.activation(g1t[:], bank(6), AF.Tanh, bias=t_b1g[:])
                ifo1 = gpool.tile([H, 1536], BF16, name="ifo1")
                nc.scalar.activation(ifo1[:], psum[:, 1536:3072], AF.Sigmoid)
                v1 = gpool.tile([H, 512], BF16, name="v1")
                nc.vector.tensor_mul(v1[:], ifo1[:, 0:512], g1t[:])
                u1 = gpool.tile([H, 512], BF16, name="u1")
                nc.vector.tensor_mul(u1[:], ifo1[:, 512:1024], c1[:])
                nc.vector.tensor_add(c1[:], u1[:], v1[:])
                tc1 = gpool.tile([H, 512], BF16, name="tc1")
                nc.scalar.activation(tc1[:], c1[:], AF.Tanh)
                nc.vector.tensor_mul(h1[:], ifo1[:, 1024:1536], tc1[:])
            if N_WARM_DUMMIES and t > 0:
                warm_dummies(N_WARM_DUMMIES)

        # encode epilogue: L1 cell T-1 (uses final h0 and the last x tile's
        # ones rows for bias)
        xlast = x_tiles[(T - 1) % 2]
        wave_a_l1(xlast)
        l1_cell(h0, xlast)

        # ---------------- decode: steps 0..PRED-1 ----------------------------
        for s in range(PRED):
            if s == 0:
                # x-part from the last input timestep (same tile as encode
                # slot T-1), row-tiled K=9 + bias
                wave_a_l0(xlast)
            else:
                # folded input path: gates_x = W0r @ r_aug (K=65, incl. bias)
                for k, gname in enumerate(CHUNKS):
                    nc.tensor.matmul(
                        bank(L0_BANK[gname]),
                        t_w0r[:, 128 * k : 128 * (k + 1)],
                        r_aug[:],
                        start=True,
                        stop=False,
                    )
            wave_a_l1(xlast)  # K=1 bias rides xlast's ones rows every step
            l0_cell(None)
            l1_cell(h0, xlast)

            # FC1 + ReLU (BN folded into weights; bias via activation bias)
            nc.tensor.matmul(bank(6)[0:FC_H, :], t_wfc1[:], h1[:], start=True, stop=True)
            nc.scalar.activation(
                r_aug[0:FC_H, :], bank(6)[0:FC_H, :], AF.Relu, bias=t_b1p[:]
            )
            # FC2 (output only) + bias via copy activation
            nc.tensor.matmul(
                bank(7)[0:OUT, :], t_wfc2[:], r_aug[0:FC_H, :], start=True, stop=True
            )
            y_sb = opool.tile([OUT, B_LOC], F32, name="y_sb")
            nc.scalar.activation(y_sb[:], bank(7)[0:OUT, :], AF.Identity, bias=t_b2[:])
            nc.sync.dma_start(d_y.ap()[s], y_sb[:])

    nc.compile()
    return nc


def _host_prep(inputs):
    """Build per-core input maps from the full problem inputs."""
    bf = ml_dtypes.bfloat16
    x = np.asarray(inputs["x"], np.float32)
    Bfull, T, _ = x.shape
    W0x = np.asarray(inputs["Wih0"], np.float32)
    W0h = np.asarray(inputs["Whh0"], np.float32)
    b0 = np.asarray(inputs["bih0"], np.float32) + np.asarray(inputs["bhh0"], np.float32)
    W1i = np.asarray(inputs["Wih1"], np.float32)
    W1h = np.asarray(inputs["Whh1"], np.float32)
    b1 = np.asarray(inputs["bih1"], np.float32) + np.asarray(inputs["bhh1"], np.float32)
    gamma = np.asarray(inputs["bn_gamma"], np.float32)
    beta = np.asarray(inputs["bn_beta"], np.float32)
    mean = np.asarray(inputs["bn_mean"], np.float32)
    var = np.asarray(inputs["bn_var"], np.float32)
    W1 = np.asarray(inputs["W1"], np.float32)
    b1f = np.asarray(inputs["b1"], np.float32)
    W2 = np.asarray(inputs["W2"], np.float32)
    b2 = np.asarray(inputs["b2"], np.float32)
    Wf = np.asarray(inputs["Wf"], np.float32)
    bfb = np.asarray(inputs["bf"], np.float32)

    # BN fold into FC1
    scale = gamma / np.sqrt(var + BN_EPS)
    W1p = W1 * scale[None, :]
    b1p = b1f + W1 @ (beta - mean * scale)

    # feedback folds
    Wf2 = Wf @ W2  # [8, 64]
    bff = Wf @ b2 + bfb  # [8]
    W0r = W0x @ Wf2  # [4H, 64]
    b0r = W0x @ bff + b0  # [4H]

    # shared weight tiles
    w0x4 = np.zeros((128, 128), np.float32)
    b14 = np.zeros((128, 128), np.float32)
    for k, gname in enumerate(CHUNKS):
        rows = GATE_ROWS[gname]
        w0x4[32 * k, :] = b0[rows]
        w0x4[32 * k + 1 : 32 * k + 9, :] = W0x[rows].T
        if gname != "g":
            b14[32 * k, :] = b1[rows]

    def lhsT_of(W):
        out = np.zeros((H, 512), np.float32)
        for k, gname in enumerate(CHUNKS):
            out[:, 128 * k : 128 * (k + 1)] = W[GATE_ROWS[gname]].T
        return out

    w0r = np.zeros((FC_H + 1, 512), np.float32)
    for k, gname in enumerate(CHUNKS):
        rows = GATE_ROWS[gname]
        w0r[0:FC_H, 128 * k : 128 * (k + 1)] = W0r[rows].T
        w0r[FC_H, 128 * k : 128 * (k + 1)] = b0r[rows]

    shared = {
        "w0x4": w0x4.astype(bf),
        "b14": b14.astype(bf),
        "w0h": lhsT_of(W0h).astype(bf),
        "w1i": lhsT_of(W1i).astype(bf),
        "w1h": lhsT_of(W1h).astype(bf),
        "wfc1": W1p.T.astype(bf),
        "b1p": b1p.reshape(FC_H, 1).astype(np.float32),
        "wfc2": W2.T.astype(bf),
        "b2": b2.reshape(OUT, 1).astype(np.float32),
        "w0r": w0r.astype(bf),
        "b1g": b1[GATE_ROWS["g"]].reshape(H, 1).astype(np.float32),
    }

    in_maps = []
    bper = Bfull // N_CORES
    for ci in range(N_CORES):
        xc = x[ci * bper : (ci + 1) * bper]  # [B_loc, T, 8]
        xt = np.ascontiguousarray(xc.transpose(1, 2, 0))  # [T, 8, B_loc]
        x_enc = np.zeros((T, 128, bper), np.float32)
        for k in range(4):
            x_enc[:, 32 * k, :] = 1.0
            x_enc[:, 32 * k + 1 : 32 * k + 9, :] = xt
        m = dict(shared)
        m["x_enc"] = x_enc.astype(bf)
        in_maps.append(m)
    return in_maps


def kernel(**inputs) -> np.ndarray:
    x = np.asarray(inputs["x"])
    Bfull, T, _ = x.shape
    PRED = int(inputs["prediction_steps"])
    key = (T, PRED)
    if key not in _CACHE:
        _CACHE[key] = _build_program(T, PRED)
    nc = _CACHE[key]

    in_maps = _host_prep(inputs)
    trace = os.environ.get("KERNEL_TRACE", "0") == "1"
    if trace:
        try:
            from harness import install_ntff_hook

            install_ntff_hook()
        except Exception:
            trace = False
    res = run_bass_kernel_spmd(
        nc,
        in_maps,
        core_ids=list(range(N_CORES)),
        trace=trace,
        tmpdir=os.environ.get("KERNEL_TRACE_DIR") or None,
    )
    if trace and res.exec_time_ns is not None:
        print(f"HW exec time: {res.exec_time_ns} ns")

    bper = Bfull // N_CORES
    out = np.empty((Bfull, PRED, OUT), np.float32)
    for ci in range(N_CORES):
        y = res.results[ci]["y"]  # [PRED, OUT, B_loc]
        out[ci * bper : (ci + 1) * bper] = y.transpose(2, 0, 1)
    return out


# revision 9
# speedup vs baseline: 1.0179x; 1.0085x over previous
# Trainium2 Bass kernel for nn_AutoregressiveLSTM (2-layer LSTM encode over
# T=512 steps + 64 autoregressive decode steps with BN+FC+feedback).
#
# Strategy (per core, batch-sharded 8 ways, B_loc=512):
#   - States/gates live as [H or 4H on partitions, B on free] tiles.
#   - Weights are stationary lhsT operands (bf16); h is the moving operand.
#   - Gate PSUM bank layout: [i0, f0, o0, i1, f1, o1, g0, g1] so one
#     bank-spanning Sigmoid covers i/f/o of a layer and one Tanh covers g.
#   - L0 input contribution + bias: K=9 row-tiled matmuls (4 concurrent 32-row
#     PE groups) against a host-prepped x tile that carries a ones-row per
#     group (bias rides the matmul). L1 bias: K=1 row-tiled matmuls vs the
#     same ones rows.
#   - Encode runs L1 lagged one slot behind L0 so both layers' engine work
#     overlaps inside a slot.
#   - Decode folds BN into FC1 and folds (feedback linear ∘ FC2 ∘ Wih0) into
#     a single K=65 matmul from the ReLU activations, removing the feedback
#     path from the serial chain. FC2 itself only feeds the output DMA.
#   - bf16 everywhere on-chip except PSUM accumulation (fp32) and the final
#     y copy (fp32): the LSTM recurrence is contractive, bf16 error stays
#     ~0.5% rms (measured) instead of accumulating.
import os
import sys

import numpy as np

if "/opt/trn_rl_repo" not in sys.path:
    sys.path.insert(0, "/opt/trn_rl_repo")

import ml_dtypes  # noqa: E402
import concourse.tile as tile  # noqa: E402
from concourse import bacc, mybir  # noqa: E402
from concourse.bass_utils import run_bass_kernel_spmd  # noqa: E402

F32 = mybir.dt.float32
BF16 = mybir.dt.bfloat16
AF = mybir.ActivationFunctionType

N_CORES = 8
H = 128
IN = 8
OUT = 4
FC_H = 64
BN_EPS = 1e-5
B_LOC = 512

# gate row ranges in the PyTorch weight layout (i, f, g, o)
GATE_ROWS = {
    "i": slice(0, H),
    "f": slice(H, 2 * H),
    "g": slice(2 * H, 3 * H),
    "o": slice(3 * H, 4 * H),
}
# chunk k (PE row-group k, lhsT column block k) holds gate CHUNKS[k]
CHUNKS = ["i", "f", "o", "g"]
L0_BANK = {"i": 0, "f": 1, "o": 2, "g": 6}
L1_BANK = {"i": 3, "f": 4, "o": 5, "g": 6}
N_WARM_DUMMIES = int(os.environ.get("N_WARM_DUMMIES", "12"))
N_WARM_DUMMIES_DEC = int(os.environ.get("N_WARM_DUMMIES_DEC", "8"))
SPLIT_SIG_ENC = os.environ.get("SPLIT_SIG_ENC", "0") == "1"
SPLIT_SIG_DEC = os.environ.get("SPLIT_SIG_DEC", "1") == "1"

_CACHE = {}


def _build_program(T, PRED):
    nc = bacc.Bacc(
        "TRN2",
        target_bir_lowering=False,
        debug=False,
        enable_asserts=False,
        num_devices=N_CORES,
    )

    d_x = nc.dram_tensor("x_enc", (T, 128, B_LOC), BF16, kind="ExternalInput")
    d_w0x4 = nc.dram_tensor("w0x4", (128, 128), BF16, kind="ExternalInput")
    d_b14 = nc.dram_tensor("b14", (128, 128), BF16, kind="ExternalInput")
    d_w0h = nc.dram_tensor("w0h", (H, 512), BF16, kind="ExternalInput")
    d_w1i = nc.dram_tensor("w1i", (H, 512), BF16, kind="ExternalInput")
    d_w1h = nc.dram_tensor("w1h", (H, 512), BF16, kind="ExternalInput")
    d_wfc1 = nc.dram_tensor("wfc1", (H, FC_H), BF16, kind="ExternalInput")
    d_b1p = nc.dram_tensor("b1p", (FC_H, 1), F32, kind="ExternalInput")
    d_wfc2 = nc.dram_tensor("wfc2", (FC_H, OUT), BF16, kind="ExternalInput")
    d_b2 = nc.dram_tensor("b2", (OUT, 1), F32, kind="ExternalInput")
    d_w0r = nc.dram_tensor("w0r", (FC_H + 1, 512), BF16, kind="ExternalInput")
    d_b1g = nc.dram_tensor("b1g", (H, 1), F32, kind="ExternalInput")
    d_y = nc.dram_tensor("y", (PRED, OUT, B_LOC), F32, kind="ExternalOutput")

    from contextlib import ExitStack

    with tile.TileContext(nc) as tc, ExitStack() as ctx:
        wpool = ctx.enter_context(tc.tile_pool(name="w", bufs=1))
        spool = ctx.enter_context(tc.tile_pool(name="s", bufs=1))
        gpool = ctx.enter_context(tc.tile_pool(name="g", bufs=2))
        xpool = ctx.enter_context(tc.tile_pool(name="x", bufs=6))
        opool = ctx.enter_context(tc.tile_pool(name="o", bufs=3))
        ppool = ctx.enter_context(tc.tile_pool(name="p", bufs=1, space="PSUM"))

        t_w0x4 = wpool.tile([128, 128], BF16)
        nc.sync.dma_start(t_w0x4[:], d_w0x4.ap())
        t_b14 = wpool.tile([128, 128], BF16)
        nc.sync.dma_start(t_b14[:], d_b14.ap())
        t_w0h = wpool.tile([H, 512], BF16)
        nc.sync.dma_start(t_w0h[:], d_w0h.ap())
        t_w1i = wpool.tile([H, 512], BF16)
        nc.sync.dma_start(t_w1i[:], d_w1i.ap())
        t_w1h = wpool.tile([H, 512], BF16)
        nc.sync.dma_start(t_w1h[:], d_w1h.ap())
        t_wfc1 = wpool.tile([H, FC_H], BF16)
        nc.sync.dma_start(t_wfc1[:], d_wfc1.ap())
        t_b1p = wpool.tile([FC_H, 1], F32)
        nc.sync.dma_start(t_b1p[:], d_b1p.ap())
        t_wfc2 = wpool.tile([FC_H, OUT], BF16)
        nc.sync.dma_start(t_wfc2[:], d_wfc2.ap())
        t_b2 = wpool.tile([OUT, 1], F32)
        nc.sync.dma_start(t_b2[:], d_b2.ap())
        t_w0r = wpool.tile([FC_H + 1, 512], BF16)
        nc.sync.dma_start(t_w0r[:], d_w0r.ap())
        t_b1g = wpool.tile([H, 1], F32)
        nc.sync.dma_start(t_b1g[:], d_b1g.ap())

        h0 = spool.tile([H, B_LOC], BF16)
        c0 = spool.tile([H, B_LOC], BF16)
        h1 = spool.tile([H, B_LOC], BF16)
        c1 = spool.tile([H, B_LOC], BF16)
        for t_ in (h0, c0, h1, c1):
            nc.vector.memset(t_[:], 0.0)

        # r_aug: ReLU activations (rows 0..63) + ones row 64 for decode folds
        r_aug = spool.tile([FC_H + 1, B_LOC], BF16)
        nc.vector.memset(r_aug[64:65, :], 1.0)

        psum = ppool.tile([128, 8 * 512], F32)

        def bank(b):
            return psum[:, 512 * b : 512 * (b + 1)]

        def gate_acts(layer, split_sig, b1g_bias=False):
            """Gate activations for one cell (reads the layer's PSUM banks,
            writes bf16 SBUF tiles). Returns (i_ap, f_ap, o_ap, g_ap)."""
            base = 0 if layer == 0 else 1536
            gt = gpool.tile([H, 512], BF16, name=f"gt{layer}")
            if b1g_bias:
                nc.scalar.activation(gt[:], bank(6), AF.Tanh, bias=t_b1g[:])
            else:
                nc.scalar.activation(gt[:], bank(6), AF.Tanh)
            if split_sig:
                ift = gpool.tile([H, 1024], BF16, name=f"ift{layer}")
                nc.scalar.activation(ift[:], psum[:, base : base + 1024], AF.Sigmoid)
                ot = gpool.tile([H, 512], BF16, name=f"ot{layer}")
                nc.scalar.activation(ot[:], psum[:, base + 1024 : base + 1536], AF.Sigmoid)
                return ift[:, 0:512], ift[:, 512:1024], ot[:], gt[:]
            ifo = gpool.tile([H, 1536], BF16, name=f"ifo{layer}")
            nc.scalar.activation(ifo[:], psum[:, base : base + 1536], AF.Sigmoid)
            return ifo[:, 0:512], ifo[:, 512:1024], ifo[:, 1024:1536], gt[:]

        def cell_update(layer, aps, c_st, h_st):
            """DVE c/h update chain from gate activation tiles."""
            i_ap, f_ap, o_ap, g_ap = aps
            v = gpool.tile([H, 512], BF16, name=f"v{layer}")
            nc.vector.tensor_mul(v[:], i_ap, g_ap)
            u = gpool.tile([H, 512], BF16, name=f"u{layer}")
            nc.vector.tensor_mul(u[:], f_ap, c_st[:])
            nc.vector.tensor_add(c_st[:], u[:], v[:])
            tcx = gpool.tile([H, 512], BF16, name=f"tcx{layer}")
            nc.scalar.activation(tcx[:], c_st[:], AF.Tanh)
            nc.vector.tensor_mul(h_st[:], o_ap, tcx[:])

        def pointwise(layer, c_st, h_st, split_sig, b1g_bias=False):
            cell_update(layer, gate_acts(layer, split_sig, b1g_bias), c_st, h_st)

        def l0_cell(xt):
            """L0 gates already prefilled with x-part+bias into L0 banks by
            the caller; this adds the recurrent part and runs the pointwise
            chain. Updates h0/c0."""
            for gname in ("g", "i", "f", "o"):
                k = CHUNKS.index(gname)
                nc.tensor.matmul(
                    bank(L0_BANK[gname]),
                    t_w0h[:, 128 * k : 128 * (k + 1)],
                    h0[:],
                    start=False,
                    stop=True,
                )
            pointwise(0, c0, h0, SPLIT_SIG_DEC)

        def l1_cell(h0_src, xt_ones):
            """L1 gates: i/f/o bias prefilled into L1 banks; g bias emitted
            here (bank 6 becomes free once tanh_g0 has read g0). Adds input
            and recurrent parts, runs pointwise chain. Updates h1/c1."""
            for gname in ("g", "i", "f", "o"):
                k = CHUNKS.index(gname)
                nc.tensor.matmul(
                    bank(L1_BANK[gname]),
                    t_w1i[:, 128 * k : 128 * (k + 1)],
                    h0_src[:],
                    start=(gname == "g"),
                    stop=False,
                )
                nc.tensor.matmul(
                    bank(L1_BANK[gname]),
                    t_w1h[:, 128 * k : 128 * (k + 1)],
                    h1[:],
                    start=False,
                    stop=True,
                )
            pointwise(1, c1, h1, SPLIT_SIG_DEC, b1g_bias=True)

        def wave_a_l0(xt):
            # row-tiled K=9 x-part (+ bias via ones row) into L0 banks
            for k, gname in enumerate(CHUNKS):
                nc.tensor.matmul(
                    bank(L0_BANK[gname]),
                    t_w0x4[32 * k : 32 * k + 9, :],
                    xt[32 * k : 32 * k + 9, :],
                    start=True,
                    stop=False,
                    tile_position=(32 * k, 0),
                )

        def wave_a_l1(xt, gates=("i", "f", "o")):
            # row-tiled K=1 L1-bias (vs ones rows of xt) into L1 banks.
            # The g-gate bias shares bank 6 with g0, so it is emitted
            # separately after tanh_g0 has consumed g0.
            for gname in gates:
                k = CHUNKS.index(gname)
                nc.tensor.matmul(
                    bank(L1_BANK[gname]),
                    t_b14[32 * k : 32 * k + 1, :],
                    xt[32 * k : 32 * k + 1, :],
                    start=True,
                    stop=False,
                    tile_position=(32 * k, 0),
                )

        def warm_dummies(n):
            # keep-warm matmuls into the sacrificial bank 7: HAM re-throttles
            # the PE clock to 1.2 GHz after idle windows; these fill the gaps.
            for _ in range(n):
                nc.tensor.matmul(
                    bank(7), t_b14[0:1, :], t_w0h[0:1, :], start=True, stop=True
                )

        # ---------------- encode: slots 0..T-1 (L1 lagged by 1) -------------
        x_tiles = [None, None]  # remember last x tile for decode step 0
        for t in range(T):
            xt = xpool.tile([128, B_LOC], BF16, name="xt")
            nc.sync.dma_start(xt[:], d_x.ap()[t])
            x_tiles[t % 2] = xt

            wave_a_l0(xt)
            if t > 0:
                wave_a_l1(xt)  # i/f/o biases for lagged L1 cell t-1
            for gname in ("g", "i", "f", "o"):
                k = CHUNKS.index(gname)
                nc.tensor.matmul(
                    bank(L0_BANK[gname]),
                    t_w0h[:, 128 * k : 128 * (k + 1)],
                    h0[:],
                    start=False,
                    stop=True,
                )
            # L0 gate activations (frees banks 6, 0-2 for L1 / next slot)
            aps0 = gate_acts(0, SPLIT_SIG_ENC)
            if t > 0:
                # lagged L1 cell t-1: bank 6 is reused for g1 after tanh_g0
                # (ih1_g restarts the accumulation); the g-bias is applied via
                # tanh_g1's per-partition bias operand instead of a matmul.
                # ih1/hh1 read h0_{t-1} / h1_{t-2} (emitted before the h0/h1
                # updates below, so Tile sequences them on the old values)
                for gname in ("g", "i", "f", "o"):
                    k = CHUNKS.index(gname)
                    nc.tensor.matmul(
                        bank(L1_BANK[gname]),
                        t_w1i[:, 128 * k : 128 * (k + 1)],
                        h0[:],
                        start=(gname == "g"),
                        stop=False,
                    )
                    nc.tensor.matmul(
                        bank(L1_BANK[gname]),
                        t_w1h[:, 128 * k : 128 * (k + 1)],
                        h1[:],
                        start=False,
                        stop=True,
                    )
            # L0 c/h update — emitted after the L1 matmuls above so those
            # read h0_{t-1} (Tile sequences the h0 overwrite behind them)
            cell_update(0, aps0, c0, h0)
            if t > 0:
                # L1 pointwise chain (updates h1, c1) for lagged cell t-1
                pointwise(1, c1, h1, SPLIT_SIG_ENC, b1g_bias=True)
            if N_WARM_DUMMIES and t > 0:
                warm_dummies(N_WARM_DUMMIES)

        # encode epilogue: L1 cell T-1 (uses final h0 and the last x tile's
        # ones rows for bias)
        xlast = x_tiles[(T - 1) % 2]
        wave_a_l1(xlast)
        l1_cell(h0, xlast)

        # ---------------- decode: steps 0..PRED-1 ----------------------------
        for s in range(PRED):
            if s == 0:
                # x-part from the last input timestep (same tile as encode
                # slot T-1), row-tiled K=9 + bias
                wave_a_l0(xlast)
            else:
                # folded input path: gates_x = W0r @ r_aug (K=65, incl. bias)
                for k, gname in enumerate(CHUNKS):
                    nc.tensor.matmul(
                        bank(L0_BANK[gname]),
                        t_w0r[:, 128 * k : 128 * (k + 1)],
                        r_aug[:],
                        start=True,
                        stop=False,
                    )
            wave_a_l1(xlast)  # K=1 bias rides xlast's ones rows every step
            l0_cell(None)
            l1_cell(h0, xlast)

            # FC1 + ReLU (BN folded into weights; bias via activation bias)
            nc.tensor.matmul(bank(0)[0:FC_H, :], t_wfc1[:], h1[:], start=True, stop=True)
            nc.scalar.activation(
                r_aug[0:FC_H, :], bank(0)[0:FC_H, :], AF.Relu, bias=t_b1p[:]
            )
            # FC2 (output only) + bias via copy activation
            nc.tensor.matmul(
                bank(1)[0:OUT, :], t_wfc2[:], r_aug[0:FC_H, :], start=True, stop=True
            )
            y_sb = opool.tile([OUT, B_LOC], F32, name="y_sb")
            nc.scalar.activation(y_sb[:], bank(1)[0:OUT, :], AF.Identity, bias=t_b2[:])
            nc.sync.dma_start(d_y.ap()[s], y_sb[:])
            if N_WARM_DUMMIES_DEC:
                warm_dummies(N_WARM_DUMMIES_DEC)

    nc.compile()
    return nc


def _host_prep(inputs):
    """Build per-core input maps from the full problem inputs."""
    bf = ml_dtypes.bfloat16
    x = np.asarray(inputs["x"], np.float32)
    Bfull, T, _ = x.shape
    W0x = np.asarray(inputs["Wih0"], np.float32)
    W0h = np.asarray(inputs["Whh0"], np.float32)
    b0 = np.asarray(inputs["bih0"], np.float32) + np.asarray(inputs["bhh0"], np.float32)
    W1i = np.asarray(inputs["Wih1"], np.float32)
    W1h = np.asarray(inputs["Whh1"], np.float32)
    b1 = np.asarray(inputs["bih1"], np.float32) + np.asarray(inputs["bhh1"], np.float32)
    gamma = np.asarray(inputs["bn_gamma"], np.float32)
    beta = np.asarray(inputs["bn_beta"], np.float32)
    mean = np.asarray(inputs["bn_mean"], np.float32)
    var = np.asarray(inputs["bn_var"], np.float32)
    W1 = np.asarray(inputs["W1"], np.float32)
    b1f = np.asarray(inputs["b1"], np.float32)
    W2 = np.asarray(inputs["W2"], np.float32)
    b2 = np.asarray(inputs["b2"], np.float32)
    Wf = np.asarray(inputs["Wf"], np.float32)
    bfb = np.asarray(inputs["bf"], np.float32)

    # BN fold into FC1
    scale = gamma / np.sqrt(var + BN_EPS)
    W1p = W1 * scale[None, :]
    b1p = b1f + W1 @ (beta - mean * scale)

    # feedback folds
    Wf2 = Wf @ W2  # [8, 64]
    bff = Wf @ b2 + bfb  # [8]
    W0r = W0x @ Wf2  # [4H, 64]
    b0r = W0x @ bff + b0  # [4H]

    # shared weight tiles
    w0x4 = np.zeros((128, 128), np.float32)
    b14 = np.zeros((128, 128), np.float32)
    for k, gname in enumerate(CHUNKS):
        rows = GATE_ROWS[gname]
        w0x4[32 * k, :] = b0[rows]
        w0x4[32 * k + 1 : 32 * k + 9, :] = W0x[rows].T
        if gname != "g":
            b14[32 * k, :] = b1[rows]

    def lhsT_of(W):
        out = np.zeros((H, 512), np.float32)
        for k, gname in enumerate(CHUNKS):
            out[:, 128 * k : 128 * (k + 1)] = W[GATE_ROWS[gname]].T
        return out

    w0r = np.zeros((FC_H + 1, 512), np.float32)
    for k, gname in enumerate(CHUNKS):
        rows = GATE_ROWS[gname]
        w0r[0:FC_H, 128 * k : 128 * (k + 1)] = W0r[rows].T
        w0r[FC_H, 128 * k : 128 * (k + 1)] = b0r[rows]

    shared = {
        "w0x4": w0x4.astype(bf),
        "b14": b14.astype(bf),
        "w0h": lhsT_of(W0h).astype(bf),
        "w1i": lhsT_of(W1i).astype(bf),
        "w1h": lhsT_of(W1h).astype(bf),
        "wfc1": W1p.T.astype(bf),
        "b1p": b1p.reshape(FC_H, 1).astype(np.float32),
        "wfc2": W2.T.astype(bf),
        "b2": b2.reshape(OUT, 1).astype(np.float32),
        "w0r": w0r.astype(bf),
        "b1g": b1[GATE_ROWS["g"]].reshape(H, 1).astype(np.float32),
    }

    in_maps = []
    bper = Bfull // N_CORES
    for ci in range(N_CORES):
        xc = x[ci * bper : (ci + 1) * bper]  # [B_loc, T, 8]
        xt = np.ascontiguousarray(xc.transpose(1, 2, 0))  # [T, 8, B_loc]
        x_enc = np.zeros((T, 128, bper), np.float32)
        for k in range(4):
            x_enc[:, 32 * k, :] = 1.0
            x_enc[:, 32 * k + 1 : 32 * k + 9, :] = xt
        m = dict(shared)
        m["x_enc"] = x_enc.astype(bf)
        in_maps.append(m)
    return in_maps


def kernel(**inputs) -> np.ndarray:
    x = np.asarray(inputs["x"])
    Bfull, T, _ = x.shape
    PRED = int(inputs["prediction_steps"])
    key = (T, PRED)
    if key not in _CACHE:
        _CACHE[key] = _build_program(T, PRED)
    nc = _CACHE[key]

    in_maps = _host_prep(inputs)
    trace = os.environ.get("KERNEL_TRACE", "0") == "1"
    if trace:
        try:
            from harness import install_ntff_hook

            install_ntff_hook()
        except Exception:
            trace = False
    res = run_bass_kernel_spmd(
        nc,
        in_maps,
        core_ids=list(range(N_CORES)),
        trace=trace,
        tmpdir=os.environ.get("KERNEL_TRACE_DIR") or None,
    )
    if trace and res.exec_time_ns is not None:
        print(f"HW exec time: {res.exec_time_ns} ns")

    bper = Bfull // N_CORES
    out = np.empty((Bfull, PRED, OUT), np.float32)
    for ci in range(N_CORES):
        y = res.results[ci]["y"]  # [PRED, OUT, B_loc]
        out[ci * bper : (ci + 1) * bper] = y.transpose(2, 0, 1)
    return out


# revision 10
# speedup vs baseline: 1.0226x; 1.0047x over previous
# Trainium2 Bass kernel for nn_AutoregressiveLSTM (2-layer LSTM encode over
# T=512 steps + 64 autoregressive decode steps with BN+FC+feedback).
#
# Strategy (per core, batch-sharded 8 ways, B_loc=512):
#   - States/gates live as [H or 4H on partitions, B on free] tiles.
#   - Weights are stationary lhsT operands (bf16); h is the moving operand.
#   - Gate PSUM bank layout: [i0, f0, o0, i1, f1, o1, g0, g1] so one
#     bank-spanning Sigmoid covers i/f/o of a layer and one Tanh covers g.
#   - L0 input contribution + bias: K=9 row-tiled matmuls (4 concurrent 32-row
#     PE groups) against a host-prepped x tile that carries a ones-row per
#     group (bias rides the matmul). L1 bias: K=1 row-tiled matmuls vs the
#     same ones rows.
#   - Encode runs L1 lagged one slot behind L0 so both layers' engine work
#     overlaps inside a slot.
#   - Decode folds BN into FC1 and folds (feedback linear ∘ FC2 ∘ Wih0) into
#     a single K=65 matmul from the ReLU activations, removing the feedback
#     path from the serial chain. FC2 itself only feeds the output DMA.
#   - bf16 everywhere on-chip except PSUM accumulation (fp32) and the final
#     y copy (fp32): the LSTM recurrence is contractive, bf16 error stays
#     ~0.5% rms (measured) instead of accumulating.
import os
import sys

import numpy as np

if "/opt/trn_rl_repo" not in sys.path:
    sys.path.insert(0, "/opt/trn_rl_repo")

import ml_dtypes  # noqa: E402
import concourse.tile as tile  # noqa: E402
from concourse import bacc, mybir  # noqa: E402
from concourse.bass_utils import run_bass_kernel_spmd  # noqa: E402

F32 = mybir.dt.float32
BF16 = mybir.dt.bfloat16
AF = mybir.ActivationFunctionType

N_CORES = 8
H = 128
IN = 8
OUT = 4
FC_H = 64
BN_EPS = 1e-5
B_LOC = 512

# gate row ranges in the PyTorch weight layout (i, f, g, o)
GATE_ROWS = {
    "i": slice(0, H),
    "f": slice(H, 2 * H),
    "g": slice(2 * H, 3 * H),
    "o": slice(3 * H, 4 * H),
}
# chunk k (PE row-group k, lhsT column block k) holds gate CHUNKS[k]
CHUNKS = ["i", "f", "o", "g"]
L0_BANK = {"i": 0, "f": 1, "o": 2, "g": 6}
L1_BANK = {"i": 3, "f": 4, "o": 5, "g": 6}
N_WARM_DUMMIES = int(os.environ.get("N_WARM_DUMMIES", "12"))
N_WARM_DUMMIES_DEC = int(os.environ.get("N_WARM_DUMMIES_DEC", "8"))
SPLIT_SIG_ENC = os.environ.get("SPLIT_SIG_ENC", "0") == "1"
SPLIT_SIG_DEC = os.environ.get("SPLIT_SIG_DEC", "1") == "1"

_CACHE = {}


def _build_program(T, PRED):
    nc = bacc.Bacc(
        "TRN2",
        target_bir_lowering=False,
        debug=False,
        enable_asserts=False,
        num_devices=N_CORES,
    )

    d_x = nc.dram_tensor("x_enc", (T, 128, B_LOC), BF16, kind="ExternalInput")
    d_w0x4 = nc.dram_tensor("w0x4", (128, 128), BF16, kind="ExternalInput")
    d_b14 = nc.dram_tensor("b14", (128, 128), BF16, kind="ExternalInput")
    d_w0h = nc.dram_tensor("w0h", (H, 512), BF16, kind="ExternalInput")
    d_w1i = nc.dram_tensor("w1i", (H, 512), BF16, kind="ExternalInput")
    d_w1h = nc.dram_tensor("w1h", (H, 512), BF16, kind="ExternalInput")
    d_wfc1 = nc.dram_tensor("wfc1", (H, FC_H), BF16, kind="ExternalInput")
    d_b1p = nc.dram_tensor("b1p", (FC_H, 1), F32, kind="ExternalInput")
    d_wfc2 = nc.dram_tensor("wfc2", (FC_H, OUT), BF16, kind="ExternalInput")
    d_b2 = nc.dram_tensor("b2", (OUT, 1), F32, kind="ExternalInput")
    d_w0r = nc.dram_tensor("w0r", (FC_H + 1, 512), BF16, kind="ExternalInput")
    d_b1g = nc.dram_tensor("b1g", (H, 1), F32, kind="ExternalInput")
    d_y = nc.dram_tensor("y", (PRED, OUT, B_LOC), F32, kind="ExternalOutput")

    from contextlib import ExitStack

    with tile.TileContext(nc) as tc, ExitStack() as ctx:
        wpool = ctx.enter_context(tc.tile_pool(name="w", bufs=1))
        spool = ctx.enter_context(tc.tile_pool(name="s", bufs=1))
        gpool = ctx.enter_context(tc.tile_pool(name="g", bufs=2))
        xpool = ctx.enter_context(tc.tile_pool(name="x", bufs=6))
        opool = ctx.enter_context(tc.tile_pool(name="o", bufs=3))
        ppool = ctx.enter_context(tc.tile_pool(name="p", bufs=1, space="PSUM"))

        t_w0x4 = wpool.tile([128, 128], BF16)
        nc.sync.dma_start(t_w0x4[:], d_w0x4.ap())
        t_b14 = wpool.tile([128, 128], BF16)
        nc.sync.dma_start(t_b14[:], d_b14.ap())
        t_w0h = wpool.tile([H, 512], BF16)
        nc.sync.dma_start(t_w0h[:], d_w0h.ap())
        t_w1i = wpool.tile([H, 512], BF16)
        nc.sync.dma_start(t_w1i[:], d_w1i.ap())
        t_w1h = wpool.tile([H, 512], BF16)
        nc.sync.dma_start(t_w1h[:], d_w1h.ap())
        t_wfc1 = wpool.tile([H, FC_H], BF16)
        nc.sync.dma_start(t_wfc1[:], d_wfc1.ap())
        t_b1p = wpool.tile([FC_H, 1], F32)
        nc.sync.dma_start(t_b1p[:], d_b1p.ap())
        t_wfc2 = wpool.tile([FC_H, OUT], BF16)
        nc.sync.dma_start(t_wfc2[:], d_wfc2.ap())
        t_b2 = wpool.tile([OUT, 1], F32)
        nc.sync.dma_start(t_b2[:], d_b2.ap())
        t_w0r = wpool.tile([FC_H + 1, 512], BF16)
        nc.sync.dma_start(t_w0r[:], d_w0r.ap())
        t_b1g = wpool.tile([H, 1], F32)
        nc.sync.dma_start(t_b1g[:], d_b1g.ap())

        h0 = spool.tile([H, B_LOC], BF16)
        c0 = spool.tile([H, B_LOC], BF16)
        h1 = spool.tile([H, B_LOC], BF16)
        c1 = spool.tile([H, B_LOC], BF16)
        for t_ in (h0, c0, h1, c1):
            nc.vector.memset(t_[:], 0.0)

        # r_aug: ReLU activations (rows 0..63) + ones row 64 for decode folds
        r_aug = spool.tile([FC_H + 1, B_LOC], BF16)
        nc.vector.memset(r_aug[64:65, :], 1.0)

        psum = ppool.tile([128, 8 * 512], F32)

        def bank(b):
            return psum[:, 512 * b : 512 * (b + 1)]

        def gate_acts(layer, split_sig, b1g_bias=False):
            """Gate activations for one cell (reads the layer's PSUM banks,
            writes bf16 SBUF tiles). Returns (i_ap, f_ap, o_ap, g_ap)."""
            base = 0 if layer == 0 else 1536
            gt = gpool.tile([H, 512], BF16, name=f"gt{layer}")
            if b1g_bias:
                nc.scalar.activation(gt[:], bank(6), AF.Tanh, bias=t_b1g[:])
            else:
                nc.scalar.activation(gt[:], bank(6), AF.Tanh)
            if split_sig:
                ift = gpool.tile([H, 1024], BF16, name=f"ift{layer}")
                nc.scalar.activation(ift[:], psum[:, base : base + 1024], AF.Sigmoid)
                ot = gpool.tile([H, 512], BF16, name=f"ot{layer}")
                nc.scalar.activation(ot[:], psum[:, base + 1024 : base + 1536], AF.Sigmoid)
                return ift[:, 0:512], ift[:, 512:1024], ot[:], gt[:]
            ifo = gpool.tile([H, 1536], BF16, name=f"ifo{layer}")
            nc.scalar.activation(ifo[:], psum[:, base : base + 1536], AF.Sigmoid)
            return ifo[:, 0:512], ifo[:, 512:1024], ifo[:, 1024:1536], gt[:]

        def cell_update(layer, aps, c_st, h_st):
            """DVE c/h update chain from gate activation tiles."""
            i_ap, f_ap, o_ap, g_ap = aps
            v = gpool.tile([H, 512], BF16, name=f"v{layer}")
            nc.vector.tensor_mul(v[:], i_ap, g_ap)
            u = gpool.tile([H, 512], BF16, name=f"u{layer}")
            nc.vector.tensor_mul(u[:], f_ap, c_st[:])
            nc.vector.tensor_add(c_st[:], u[:], v[:])
            tcx = gpool.tile([H, 512], BF16, name=f"tcx{layer}")
            nc.scalar.activation(tcx[:], c_st[:], AF.Tanh)
            nc.vector.tensor_mul(h_st[:], o_ap, tcx[:])

        def pointwise(layer, c_st, h_st, split_sig, b1g_bias=False):
            cell_update(layer, gate_acts(layer, split_sig, b1g_bias), c_st, h_st)

        def l0_cell(xt):
            """L0 gates already prefilled with x-part+bias into L0 banks by
            the caller; this adds the recurrent part and runs the pointwise
            chain. Updates h0/c0."""
            for gname in ("g", "i", "f", "o"):
                k = CHUNKS.index(gname)
                nc.tensor.matmul(
                    bank(L0_BANK[gname]),
                    t_w0h[:, 128 * k : 128 * (k + 1)],
                    h0[:],
                    start=False,
                    stop=True,
                )
            pointwise(0, c0, h0, SPLIT_SIG_DEC)

        def l1_cell(h0_src, xt_ones):
            """L1 gates: i/f/o bias prefilled into L1 banks; g bias emitted
            here (bank 6 becomes free once tanh_g0 has read g0). Adds input
            and recurrent parts, runs pointwise chain. Updates h1/c1."""
            for gname in ("g", "i", "f", "o"):
                k = CHUNKS.index(gname)
                nc.tensor.matmul(
                    bank(L1_BANK[gname]),
                    t_w1i[:, 128 * k : 128 * (k + 1)],
                    h0_src[:],
                    start=(gname == "g"),
                    stop=False,
                )
                nc.tensor.matmul(
                    bank(L1_BANK[gname]),
                    t_w1h[:, 128 * k : 128 * (k + 1)],
                    h1[:],
                    start=False,
                    stop=True,
                )
            pointwise(1, c1, h1, SPLIT_SIG_DEC, b1g_bias=True)

        def wave_a_l0(xt, start=True, stop=False, order=CHUNKS):
            # row-tiled K=9 x-part (+ bias via ones row) into L0 banks
            for gname in order:
                k = CHUNKS.index(gname)
                nc.tensor.matmul(
                    bank(L0_BANK[gname]),
                    t_w0x4[32 * k : 32 * k + 9, :],
                    xt[32 * k : 32 * k + 9, :],
                    start=start,
                    stop=stop,
                    tile_position=(32 * k, 0),
                )

        def wave_a_l1(xt, gates=("i", "f", "o")):
            # row-tiled K=1 L1-bias (vs ones rows of xt) into L1 banks.
            # The g-gate bias shares bank 6 with g0, so it is emitted
            # separately after tanh_g0 has consumed g0.
            for gname in gates:
                k = CHUNKS.index(gname)
                nc.tensor.matmul(
                    bank(L1_BANK[gname]),
                    t_b14[32 * k : 32 * k + 1, :],
                    xt[32 * k : 32 * k + 1, :],
                    start=True,
                    stop=False,
                    tile_position=(32 * k, 0),
                )

        def warm_dummies(n):
            # keep-warm matmuls into the sacrificial bank 7: HAM re-throttles
            # the PE clock to 1.2 GHz after idle windows; these fill the gaps.
            for _ in range(n):
                nc.tensor.matmul(
                    bank(7), t_b14[0:1, :], t_w0h[0:1, :], start=True, stop=True
                )

        # ---------------- encode: slots 0..T-1 (L1 lagged by 1) -------------
        x_tiles = [None, None]  # remember last x tile for decode step 0
        for t in range(T):
            xt = xpool.tile([128, B_LOC], BF16, name="xt")
            nc.sync.dma_start(xt[:], d_x.ap()[t])
            x_tiles[t % 2] = xt

            wave_a_l0(xt)
            if t > 0:
                wave_a_l1(xt)  # i/f/o biases for lagged L1 cell t-1
            for gname in ("g", "i", "f", "o"):
                k = CHUNKS.index(gname)
                nc.tensor.matmul(
                    bank(L0_BANK[gname]),
                    t_w0h[:, 128 * k : 128 * (k + 1)],
                    h0[:],
                    start=False,
                    stop=True,
                )
            # L0 gate activations (frees banks 6, 0-2 for L1 / next slot)
            aps0 = gate_acts(0, SPLIT_SIG_ENC)
            if t > 0:
                # lagged L1 cell t-1: bank 6 is reused for g1 after tanh_g0
                # (ih1_g restarts the accumulation); the g-bias is applied via
                # tanh_g1's per-partition bias operand instead of a matmul.
                # ih1/hh1 read h0_{t-1} / h1_{t-2} (emitted before the h0/h1
                # updates below, so Tile sequences them on the old values)
                for gname in ("g", "i", "f", "o"):
                    k = CHUNKS.index(gname)
                    nc.tensor.matmul(
                        bank(L1_BANK[gname]),
                        t_w1i[:, 128 * k : 128 * (k + 1)],
                        h0[:],
                        start=(gname == "g"),
                        stop=False,
                    )
                    nc.tensor.matmul(
                        bank(L1_BANK[gname]),
                        t_w1h[:, 128 * k : 128 * (k + 1)],
                        h1[:],
                        start=False,
                        stop=True,
                    )
            # L0 c/h update — emitted after the L1 matmuls above so those
            # read h0_{t-1} (Tile sequences the h0 overwrite behind them)
            cell_update(0, aps0, c0, h0)
            if t > 0:
                # L1 pointwise chain (updates h1, c1) for lagged cell t-1
                pointwise(1, c1, h1, SPLIT_SIG_ENC, b1g_bias=True)
            if N_WARM_DUMMIES and t > 0:
                warm_dummies(N_WARM_DUMMIES)

        # encode epilogue: L1 cell T-1 (uses final h0 and the last x tile's
        # ones rows for bias)
        xlast = x_tiles[(T - 1) % 2]
        wave_a_l1(xlast)
        l1_cell(h0, xlast)

        # ---------------- decode: steps 0..PRED-1 ----------------------------
        # Fully serial feedback chain; the recurrent (h-dependent) matmuls run
        # start=True EARLY (during the previous step's tail) and the
        # input-path matmuls accumulate on top after ReLU, keeping only the
        # input path on the serial chain.
        for s in range(PRED):
            # hh0 first (start=True): consumes h0_{s-1}, can overlap step s-1
            for gname in ("g", "i", "f", "o"):
                k = CHUNKS.index(gname)
                nc.tensor.matmul(
                    bank(L0_BANK[gname]),
                    t_w0h[:, 128 * k : 128 * (k + 1)],
                    h0[:],
                    start=True,
                    stop=False,
                )
            if s == 0:
                # x-part from the last input timestep (same tile as encode
                # slot T-1), row-tiled K=9 + bias
                wave_a_l0(xlast, start=False, stop=True, order=("i", "f", "g", "o"))
            else:
                # folded input path: gates_x += W0r @ r_aug (K=65, incl. bias)
                for gname in ("i", "f", "g", "o"):
                    k = CHUNKS.index(gname)
                    nc.tensor.matmul(
                        bank(L0_BANK[gname]),
                        t_w0r[:, 128 * k : 128 * (k + 1)],
                        r_aug[:],
                        start=False,
                        stop=True,
                    )
            aps0 = gate_acts(0, SPLIT_SIG_DEC)
            # L1 early parts: i/f/o bias wave + hh1 (h1_{s-1}); the g-gate
            # recurrent mm restarts bank 6 after tanh_g0 has read g0
            wave_a_l1(xlast)
            for gname in ("g", "i", "f", "o"):
                k = CHUNKS.index(gname)
                nc.tensor.matmul(
                    bank(L1_BANK[gname]),
                    t_w1h[:, 128 * k : 128 * (k + 1)],
                    h1[:],
                    start=(gname == "g"),
                    stop=False,
                )
            cell_update(0, aps0, c0, h0)
            # ih1 consumes the fresh h0_s (emitted after the h0 update)
            for gname in ("i", "f", "g", "o"):
                k = CHUNKS.index(gname)
                nc.tensor.matmul(
                    bank(L1_BANK[gname]),
                    t_w1i[:, 128 * k : 128 * (k + 1)],
                    h0[:],
                    start=False,
                    stop=True,
                )
            aps1 = gate_acts(1, SPLIT_SIG_DEC, b1g_bias=True)
            cell_update(1, aps1, c1, h1)

            # FC1 + ReLU (BN folded into weights; bias via activation bias)
            nc.tensor.matmul(bank(0)[0:FC_H, :], t_wfc1[:], h1[:], start=True, stop=True)
            nc.scalar.activation(
                r_aug[0:FC_H, :], bank(0)[0:FC_H, :], AF.Relu, bias=t_b1p[:]
            )
            # FC2 (output only) + bias via copy activation
            nc.tensor.matmul(
                bank(1)[0:OUT, :], t_wfc2[:], r_aug[0:FC_H, :], start=True, stop=True
            )
            y_sb = opool.tile([OUT, B_LOC], F32, name="y_sb")
            nc.scalar.activation(y_sb[:], bank(1)[0:OUT, :], AF.Identity, bias=t_b2[:])
            nc.sync.dma_start(d_y.ap()[s], y_sb[:])
            if N_WARM_DUMMIES_DEC:
                warm_dummies(N_WARM_DUMMIES_DEC)

    nc.compile()
    return nc


def _host_prep(inputs):
    """Build per-core input maps from the full problem inputs."""
    bf = ml_dtypes.bfloat16
    x = np.asarray(inputs["x"], np.float32)
    Bfull, T, _ = x.shape
    W0x = np.asarray(inputs["Wih0"], np.float32)
    W0h = np.asarray(inputs["Whh0"], np.float32)
    b0 = np.asarray(inputs["bih0"], np.float32) + np.asarray(inputs["bhh0"], np.float32)
    W1i = np.asarray(inputs["Wih1"], np.float32)
    W1h = np.asarray(inputs["Whh1"], np.float32)
    b1 = np.asarray(inputs["bih1"], np.float32) + np.asarray(inputs["bhh1"], np.float32)
    gamma = np.asarray(inputs["bn_gamma"], np.float32)
    beta = np.asarray(inputs["bn_beta"], np.float32)
    mean = np.asarray(inputs["bn_mean"], np.float32)
    var = np.asarray(inputs["bn_var"], np.float32)
    W1 = np.asarray(inputs["W1"], np.float32)
    b1f = np.asarray(inputs["b1"], np.float32)
    W2 = np.asarray(inputs["W2"], np.float32)
    b2 = np.asarray(inputs["b2"], np.float32)
    Wf = np.asarray(inputs["Wf"], np.float32)
    bfb = np.asarray(inputs["bf"], np.float32)

    # BN fold into FC1
    scale = gamma / np.sqrt(var + BN_EPS)
    W1p = W1 * scale[None, :]
    b1p = b1f + W1 @ (beta - mean * scale)

    # feedback folds
    Wf2 = Wf @ W2  # [8, 64]
    bff = Wf @ b2 + bfb  # [8]
    W0r = W0x @ Wf2  # [4H, 64]
    b0r = W0x @ bff + b0  # [4H]

    # shared weight tiles
    w0x4 = np.zeros((128, 128), np.float32)
    b14 = np.zeros((128, 128), np.float32)
    for k, gname in enumerate(CHUNKS):
        rows = GATE_ROWS[gname]
        w0x4[32 * k, :] = b0[rows]
        w0x4[32 * k + 1 : 32 * k + 9, :] = W0x[rows].T
        if gname != "g":
            b14[32 * k, :] = b1[rows]

    def lhsT_of(W):
        out = np.zeros((H, 512), np.float32)
        for k, gname in enumerate(CHUNKS):
            out[:, 128 * k : 128 * (k + 1)] = W[GATE_ROWS[gname]].T
        return out

    w0r = np.zeros((FC_H + 1, 512), np.float32)
    for k, gname in enumerate(CHUNKS):
        rows = GATE_ROWS[gname]
        w0r[0:FC_H, 128 * k : 128 * (k + 1)] = W0r[rows].T
        w0r[FC_H, 128 * k : 128 * (k + 1)] = b0r[rows]

    shared = {
        "w0x4": w0x4.astype(bf),
        "b14": b14.astype(bf),
        "w0h": lhsT_of(W0h).astype(bf),
        "w1i": lhsT_of(W1i).astype(bf),
        "w1h": lhsT_of(W1h).astype(bf),
        "wfc1": W1p.T.astype(bf),
        "b1p": b1p.reshape(FC_H, 1).astype(np.float32),
        "wfc2": W2.T.astype(bf),
        "b2": b2.reshape(OUT, 1).astype(np.float32),
        "w0r": w0r.astype(bf),
        "b1g": b1[GATE_ROWS["g"]].reshape(H, 1).astype(np.float32),
    }

    in_maps = []
    bper = Bfull // N_CORES
    for ci in range(N_CORES):
        xc = x[ci * bper : (ci + 1) * bper]  # [B_loc, T, 8]
        xt = np.ascontiguousarray(xc.transpose(1, 2, 0))  # [T, 8, B_loc]
        x_enc = np.zeros((T, 128, bper), np.float32)
        for k in range(4):
            x_enc[:, 32 * k, :] = 1.0
            x_enc[:, 32 * k + 1 : 32 * k + 9, :] = xt
        m = dict(shared)
        m["x_enc"] = x_enc.astype(bf)
        in_maps.append(m)
    return in_maps


def kernel(**inputs) -> np.ndarray:
    x = np.asarray(inputs["x"])
    Bfull, T, _ = x.shape
    PRED = int(inputs["prediction_steps"])
    key = (T, PRED)
    if key not in _CACHE:
        _CACHE[key] = _build_program(T, PRED)
    nc = _CACHE[key]

    in_maps = _host_prep(inputs)
    trace = os.environ.get("KERNEL_TRACE", "0") == "1"
    if trace:
        try:
            from harness import install_ntff_hook

            install_ntff_hook()
        except Exception:
            trace = False
    res = run_bass_kernel_spmd(
        nc,
        in_maps,
        core_ids=list(range(N_CORES)),
        trace=trace,
        tmpdir=os.environ.get("KERNEL_TRACE_DIR") or None,
    )
    if trace and res.exec_time_ns is not None:
        print(f"HW exec time: {res.exec_time_ns} ns")

    bper = Bfull // N_CORES
    out = np.empty((Bfull, PRED, OUT), np.float32)
    for ci in range(N_CORES):
        y = res.results[ci]["y"]  # [PRED, OUT, B_loc]
        out[ci * bper : (ci + 1) * bper] = y.transpose(2, 0, 1)
    return out


# revision 11
# speedup vs baseline: 1.2227x; 1.1956x over previous
# Trainium2 Bass kernel for nn_AutoregressiveLSTM (2-layer LSTM encode over
# T=512 steps + 64 autoregressive decode steps with BN+FC+feedback).
#
# Strategy (per core, batch-sharded 8 ways, B_loc=512):
#   - States/gates live as [H or 4H on partitions, B on free] tiles.
#   - Weights are stationary lhsT operands (bf16); h is the moving operand.
#   - Gate PSUM bank layout: [i0, f0, o0, i1, f1, o1, g0, g1] so one
#     bank-spanning Sigmoid covers i/f/o of a layer and one Tanh covers g.
#   - L0 input contribution + bias: K=9 row-tiled matmuls (4 concurrent 32-row
#     PE groups) against a host-prepped x tile that carries a ones-row per
#     group (bias rides the matmul). L1 bias: K=1 row-tiled matmuls vs the
#     same ones rows.
#   - Encode runs L1 lagged one slot behind L0 so both layers' engine work
#     overlaps inside a slot.
#   - Decode folds BN into FC1 and folds (feedback linear ∘ FC2 ∘ Wih0) into
#     a single K=65 matmul from the ReLU activations, removing the feedback
#     path from the serial chain. FC2 itself only feeds the output DMA.
#   - bf16 everywhere on-chip except PSUM accumulation (fp32) and the final
#     y copy (fp32): the LSTM recurrence is contractive, bf16 error stays
#     ~0.5% rms (measured) instead of accumulating.
import os
import sys

import numpy as np

if "/opt/trn_rl_repo" not in sys.path:
    sys.path.insert(0, "/opt/trn_rl_repo")

import ml_dtypes  # noqa: E402
import concourse.tile as tile  # noqa: E402
from concourse import bacc, mybir  # noqa: E402
from concourse.bass_utils import run_bass_kernel_spmd  # noqa: E402

F32 = mybir.dt.float32
BF16 = mybir.dt.bfloat16
AF = mybir.ActivationFunctionType

N_CORES = 8
H = 128
IN = 8
OUT = 4
FC_H = 64
BN_EPS = 1e-5
B_LOC = 512

# gate row ranges in the PyTorch weight layout (i, f, g, o)
GATE_ROWS = {
    "i": slice(0, H),
    "f": slice(H, 2 * H),
    "g": slice(2 * H, 3 * H),
    "o": slice(3 * H, 4 * H),
}
# chunk k (PE row-group k, lhsT column block k) holds gate CHUNKS[k]
CHUNKS = ["i", "f", "o", "g"]
L0_BANK = {"i": 0, "f": 1, "o": 2, "g": 6}
L1_BANK = {"i": 3, "f": 4, "o": 5, "g": 6}
N_WARM_DUMMIES = int(os.environ.get("N_WARM_DUMMIES", "12"))
N_WARM_DUMMIES_DEC = int(os.environ.get("N_WARM_DUMMIES_DEC", "8"))
SPLIT_SIG_ENC = int(os.environ.get("SPLIT_SIG_ENC", "0"))
SPLIT_SIG_DEC = int(os.environ.get("SPLIT_SIG_DEC", "2"))

_CACHE = {}


def _build_program(T, PRED):
    nc = bacc.Bacc(
        "TRN2",
        target_bir_lowering=False,
        debug=False,
        enable_asserts=False,
        num_devices=N_CORES,
    )

    d_x = nc.dram_tensor("x_enc", (T, 128, B_LOC), BF16, kind="ExternalInput")
    d_w0x4 = nc.dram_tensor("w0x4", (128, 128), BF16, kind="ExternalInput")
    d_b14 = nc.dram_tensor("b14", (128, 128), BF16, kind="ExternalInput")
    d_w0h = nc.dram_tensor("w0h", (H, 512), BF16, kind="ExternalInput")
    d_w1i = nc.dram_tensor("w1i", (H, 512), BF16, kind="ExternalInput")
    d_w1h = nc.dram_tensor("w1h", (H, 512), BF16, kind="ExternalInput")
    d_wfc1 = nc.dram_tensor("wfc1", (H, FC_H), BF16, kind="ExternalInput")
    d_b1p = nc.dram_tensor("b1p", (FC_H, 1), F32, kind="ExternalInput")
    d_wfc2 = nc.dram_tensor("wfc2", (FC_H, OUT), BF16, kind="ExternalInput")
    d_b2 = nc.dram_tensor("b2", (OUT, 1), F32, kind="ExternalInput")
    d_w0r = nc.dram_tensor("w0r", (FC_H + 1, 512), BF16, kind="ExternalInput")
    d_b1g = nc.dram_tensor("b1g", (H, 1), F32, kind="ExternalInput")
    d_y = nc.dram_tensor("y", (PRED, OUT, B_LOC), F32, kind="ExternalOutput")

    from contextlib import ExitStack

    with tile.TileContext(nc) as tc, ExitStack() as ctx:
        wpool = ctx.enter_context(tc.tile_pool(name="w", bufs=1))
        spool = ctx.enter_context(tc.tile_pool(name="s", bufs=1))
        gpool = ctx.enter_context(tc.tile_pool(name="g", bufs=2))
        xpool = ctx.enter_context(tc.tile_pool(name="x", bufs=6))
        opool = ctx.enter_context(tc.tile_pool(name="o", bufs=3))
        ppool = ctx.enter_context(tc.tile_pool(name="p", bufs=1, space="PSUM"))

        t_w0x4 = wpool.tile([128, 128], BF16)
        nc.sync.dma_start(t_w0x4[:], d_w0x4.ap())
        t_b14 = wpool.tile([128, 128], BF16)
        nc.sync.dma_start(t_b14[:], d_b14.ap())
        t_w0h = wpool.tile([H, 512], BF16)
        nc.sync.dma_start(t_w0h[:], d_w0h.ap())
        t_w1i = wpool.tile([H, 512], BF16)
        nc.sync.dma_start(t_w1i[:], d_w1i.ap())
        t_w1h = wpool.tile([H, 512], BF16)
        nc.sync.dma_start(t_w1h[:], d_w1h.ap())
        t_wfc1 = wpool.tile([H, FC_H], BF16)
        nc.sync.dma_start(t_wfc1[:], d_wfc1.ap())
        t_b1p = wpool.tile([FC_H, 1], F32)
        nc.sync.dma_start(t_b1p[:], d_b1p.ap())
        t_wfc2 = wpool.tile([FC_H, OUT], BF16)
        nc.sync.dma_start(t_wfc2[:], d_wfc2.ap())
        t_b2 = wpool.tile([OUT, 1], F32)
        nc.sync.dma_start(t_b2[:], d_b2.ap())
        t_w0r = wpool.tile([FC_H + 1, 512], BF16)
        nc.sync.dma_start(t_w0r[:], d_w0r.ap())
        t_b1g = wpool.tile([H, 1], F32)
        nc.sync.dma_start(t_b1g[:], d_b1g.ap())

        h0 = spool.tile([H, B_LOC], BF16)
        c0 = spool.tile([H, B_LOC], BF16)
        h1 = spool.tile([H, B_LOC], BF16)
        c1 = spool.tile([H, B_LOC], BF16)
        for t_ in (h0, c0, h1, c1):
            nc.vector.memset(t_[:], 0.0)

        # r_aug: ReLU activations (rows 0..63) + ones row 64 for decode folds
        r_aug = spool.tile([FC_H + 1, B_LOC], BF16)
        nc.vector.memset(r_aug[64:65, :], 1.0)

        psum = ppool.tile([128, 8 * 512], F32)

        def bank(b):
            return psum[:, 512 * b : 512 * (b + 1)]

        def gate_acts(layer, split_sig, b1g_bias=False):
            """Gate activations for one cell (reads the layer's PSUM banks,
            writes bf16 SBUF tiles). Returns (i_ap, f_ap, o_ap, g_ap)."""
            base = 0 if layer == 0 else 1536
            gt = gpool.tile([H, 512], BF16, name=f"gt{layer}")
            if b1g_bias:
                nc.scalar.activation(gt[:], bank(6), AF.Tanh, bias=t_b1g[:])
            else:
                nc.scalar.activation(gt[:], bank(6), AF.Tanh)
            if split_sig == 2:
                it = gpool.tile([H, 512], BF16, name=f"it{layer}")
                nc.scalar.activation(it[:], psum[:, base : base + 512], AF.Sigmoid)
                ft = gpool.tile([H, 512], BF16, name=f"ft{layer}")
                nc.scalar.activation(ft[:], psum[:, base + 512 : base + 1024], AF.Sigmoid)
                ot = gpool.tile([H, 512], BF16, name=f"ot{layer}")
                nc.scalar.activation(ot[:], psum[:, base + 1024 : base + 1536], AF.Sigmoid)
                return it[:], ft[:], ot[:], gt[:]
            if split_sig:
                ift = gpool.tile([H, 1024], BF16, name=f"ift{layer}")
                nc.scalar.activation(ift[:], psum[:, base : base + 1024], AF.Sigmoid)
                ot = gpool.tile([H, 512], BF16, name=f"ot{layer}")
                nc.scalar.activation(ot[:], psum[:, base + 1024 : base + 1536], AF.Sigmoid)
                return ift[:, 0:512], ift[:, 512:1024], ot[:], gt[:]
            ifo = gpool.tile([H, 1536], BF16, name=f"ifo{layer}")
            nc.scalar.activation(ifo[:], psum[:, base : base + 1536], AF.Sigmoid)
            return ifo[:, 0:512], ifo[:, 512:1024], ifo[:, 1024:1536], gt[:]

        def cell_update(layer, aps, c_st, h_st):
            """DVE c/h update chain from gate activation tiles."""
            i_ap, f_ap, o_ap, g_ap = aps
            v = gpool.tile([H, 512], BF16, name=f"v{layer}")
            nc.vector.tensor_mul(v[:], i_ap, g_ap)
            u = gpool.tile([H, 512], BF16, name=f"u{layer}")
            nc.vector.tensor_mul(u[:], f_ap, c_st[:])
            nc.vector.tensor_add(c_st[:], u[:], v[:])
            tcx = gpool.tile([H, 512], BF16, name=f"tcx{layer}")
            nc.scalar.activation(tcx[:], c_st[:], AF.Tanh)
            nc.vector.tensor_mul(h_st[:], o_ap, tcx[:])

        def pointwise(layer, c_st, h_st, split_sig, b1g_bias=False):
            cell_update(layer, gate_acts(layer, split_sig, b1g_bias), c_st, h_st)

        def l0_cell(xt):
            """L0 gates already prefilled with x-part+bias into L0 banks by
            the caller; this adds the recurrent part and runs the pointwise
            chain. Updates h0/c0."""
            for gname in ("g", "i", "f", "o"):
                k = CHUNKS.index(gname)
                nc.tensor.matmul(
                    bank(L0_BANK[gname]),
                    t_w0h[:, 128 * k : 128 * (k + 1)],
                    h0[:],
                    start=False,
                    stop=True,
                )
            pointwise(0, c0, h0, SPLIT_SIG_DEC)

        def l1_cell(h0_src, xt_ones):
            """L1 gates: i/f/o bias prefilled into L1 banks; g bias emitted
            here (bank 6 becomes free once tanh_g0 has read g0). Adds input
            and recurrent parts, runs pointwise chain. Updates h1/c1."""
            for gname in ("g", "i", "f", "o"):
                k = CHUNKS.index(gname)
                nc.tensor.matmul(
                    bank(L1_BANK[gname]),
                    t_w1i[:, 128 * k : 128 * (k + 1)],
                    h0_src[:],
                    start=(gname == "g"),
                    stop=False,
                )
                nc.tensor.matmul(
                    bank(L1_BANK[gname]),
                    t_w1h[:, 128 * k : 128 * (k + 1)],
                    h1[:],
                    start=False,
                    stop=True,
                )
            pointwise(1, c1, h1, SPLIT_SIG_DEC, b1g_bias=True)

        def wave_a_l0(xt, start=True, stop=False, order=CHUNKS):
            # row-tiled K=9 x-part (+ bias via ones row) into L0 banks
            for gname in order:
                k = CHUNKS.index(gname)
                nc.tensor.matmul(
                    bank(L0_BANK[gname]),
                    t_w0x4[32 * k : 32 * k + 9, :],
                    xt[32 * k : 32 * k + 9, :],
                    start=start,
                    stop=stop,
                    tile_position=(32 * k, 0),
                )

        def wave_a_l1(xt, gates=("i", "f", "o")):
            # row-tiled K=1 L1-bias (vs ones rows of xt) into L1 banks.
            # The g-gate bias shares bank 6 with g0, so it is emitted
            # separately after tanh_g0 has consumed g0.
            for gname in gates:
                k = CHUNKS.index(gname)
                nc.tensor.matmul(
                    bank(L1_BANK[gname]),
                    t_b14[32 * k : 32 * k + 1, :],
                    xt[32 * k : 32 * k + 1, :],
                    start=True,
                    stop=False,
                    tile_position=(32 * k, 0),
                )

        def warm_dummies(n):
            # keep-warm matmuls into the sacrificial bank 7: HAM re-throttles
            # the PE clock to 1.2 GHz after idle windows; these fill the gaps.
            for _ in range(n):
                nc.tensor.matmul(
                    bank(7), t_b14[0:1, :], t_w0h[0:1, :], start=True, stop=True
                )

        # ---------------- encode: slots 0..T-1 (L1 lagged by 1) -------------
        x_tiles = [None, None]  # remember last x tile for decode step 0
        for t in range(T):
            xt = xpool.tile([128, B_LOC], BF16, name="xt")
            nc.sync.dma_start(xt[:], d_x.ap()[t])
            x_tiles[t % 2] = xt

            wave_a_l0(xt)
            if t > 0:
                wave_a_l1(xt)  # i/f/o biases for lagged L1 cell t-1
            for gname in ("g", "i", "f", "o"):
                k = CHUNKS.index(gname)
                nc.tensor.matmul(
                    bank(L0_BANK[gname]),
                    t_w0h[:, 128 * k : 128 * (k + 1)],
                    h0[:],
                    start=False,
                    stop=True,
                )
            # L0 gate activations (frees banks 6, 0-2 for L1 / next slot)
            aps0 = gate_acts(0, SPLIT_SIG_ENC)
            if t > 0:
                # lagged L1 cell t-1: bank 6 is reused for g1 after tanh_g0
                # (ih1_g restarts the accumulation); the g-bias is applied via
                # tanh_g1's per-partition bias operand instead of a matmul.
                # ih1/hh1 read h0_{t-1} / h1_{t-2} (emitted before the h0/h1
                # updates below, so Tile sequences them on the old values)
                for gname in ("g", "i", "f", "o"):
                    k = CHUNKS.index(gname)
                    nc.tensor.matmul(
                        bank(L1_BANK[gname]),
                        t_w1i[:, 128 * k : 128 * (k + 1)],
                        h0[:],
                        start=(gname == "g"),
                        stop=False,
                    )
                    nc.tensor.matmul(
                        bank(L1_BANK[gname]),
                        t_w1h[:, 128 * k : 128 * (k + 1)],
                        h1[:],
                        start=False,
                        stop=True,
                    )
            # L0 c/h update — emitted after the L1 matmuls above so those
            # read h0_{t-1} (Tile sequences the h0 overwrite behind them)
            cell_update(0, aps0, c0, h0)
            if t > 0:
                # L1 pointwise chain (updates h1, c1) for lagged cell t-1
                pointwise(1, c1, h1, SPLIT_SIG_ENC, b1g_bias=True)
            if N_WARM_DUMMIES and t > 0:
                warm_dummies(N_WARM_DUMMIES)

        # encode epilogue: L1 cell T-1 (uses final h0 and the last x tile's
        # ones rows for bias)
        xlast = x_tiles[(T - 1) % 2]
        wave_a_l1(xlast)
        l1_cell(h0, xlast)

        # ---------------- decode: steps 0..PRED-1 ----------------------------
        # Fully serial feedback chain; the recurrent (h-dependent) matmuls run
        # start=True EARLY (during the previous step's tail) and the
        # input-path matmuls accumulate on top after ReLU, keeping only the
        # input path on the serial chain.
        for s in range(PRED):
            # hh0 first (start=True): consumes h0_{s-1}, can overlap step s-1
            for gname in ("g", "i", "f", "o"):
                k = CHUNKS.index(gname)
                nc.tensor.matmul(
                    bank(L0_BANK[gname]),
                    t_w0h[:, 128 * k : 128 * (k + 1)],
                    h0[:],
                    start=True,
                    stop=False,
                )
            if s == 0:
                # x-part from the last input timestep (same tile as encode
                # slot T-1), row-tiled K=9 + bias
                wave_a_l0(xlast, start=False, stop=True, order=("i", "f", "g", "o"))
            else:
                # folded input path: gates_x += W0r @ r_aug (K=65, incl. bias)
                for gname in ("i", "f", "g", "o"):
                    k = CHUNKS.index(gname)
                    nc.tensor.matmul(
                        bank(L0_BANK[gname]),
                        t_w0r[:, 128 * k : 128 * (k + 1)],
                        r_aug[:],
                        start=False,
                        stop=True,
                    )
            aps0 = gate_acts(0, SPLIT_SIG_DEC)
            # L1 early parts: i/f/o bias wave + hh1 (h1_{s-1}); the g-gate
            # recurrent mm restarts bank 6 after tanh_g0 has read g0
            wave_a_l1(xlast)
            for gname in ("g", "i", "f", "o"):
                k = CHUNKS.index(gname)
                nc.tensor.matmul(
                    bank(L1_BANK[gname]),
                    t_w1h[:, 128 * k : 128 * (k + 1)],
                    h1[:],
                    start=(gname == "g"),
                    stop=False,
                )
            cell_update(0, aps0, c0, h0)
            # ih1 consumes the fresh h0_s (emitted after the h0 update)
            for gname in ("i", "f", "g", "o"):
                k = CHUNKS.index(gname)
                nc.tensor.matmul(
                    bank(L1_BANK[gname]),
                    t_w1i[:, 128 * k : 128 * (k + 1)],
                    h0[:],
                    start=False,
                    stop=True,
                )
            aps1 = gate_acts(1, SPLIT_SIG_DEC, b1g_bias=True)
            cell_update(1, aps1, c1, h1)

            # FC1 + ReLU (BN folded into weights; bias via activation bias)
            nc.tensor.matmul(bank(2)[0:FC_H, :], t_wfc1[:], h1[:], start=True, stop=True)
            nc.scalar.activation(
                r_aug[0:FC_H, :], bank(2)[0:FC_H, :], AF.Relu, bias=t_b1p[:]
            )
            # FC2 (output only) + bias via copy activation
            nc.tensor.matmul(
                bank(5)[0:OUT, :], t_wfc2[:], r_aug[0:FC_H, :], start=True, stop=True
            )
            y_sb = opool.tile([OUT, B_LOC], F32, name="y_sb")
            nc.scalar.activation(y_sb[:], bank(5)[0:OUT, :], AF.Identity, bias=t_b2[:])
            nc.sync.dma_start(d_y.ap()[s], y_sb[:])
            if N_WARM_DUMMIES_DEC:
                warm_dummies(N_WARM_DUMMIES_DEC)

    nc.compile()
    return nc


def _host_prep(inputs):
    """Build per-core input maps from the full problem inputs."""
    bf = ml_dtypes.bfloat16
    x = np.asarray(inputs["x"], np.float32)
    Bfull, T, _ = x.shape
    W0x = np.asarray(inputs["Wih0"], np.float32)
    W0h = np.asarray(inputs["Whh0"], np.float32)
    b0 = np.asarray(inputs["bih0"], np.float32) + np.asarray(inputs["bhh0"], np.float32)
    W1i = np.asarray(inputs["Wih1"], np.float32)
    W1h = np.asarray(inputs["Whh1"], np.float32)
    b1 = np.asarray(inputs["bih1"], np.float32) + np.asarray(inputs["bhh1"], np.float32)
    gamma = np.asarray(inputs["bn_gamma"], np.float32)
    beta = np.asarray(inputs["bn_beta"], np.float32)
    mean = np.asarray(inputs["bn_mean"], np.float32)
    var = np.asarray(inputs["bn_var"], np.float32)
    W1 = np.asarray(inputs["W1"], np.float32)
    b1f = np.asarray(inputs["b1"], np.float32)
    W2 = np.asarray(inputs["W2"], np.float32)
    b2 = np.asarray(inputs["b2"], np.float32)
    Wf = np.asarray(inputs["Wf"], np.float32)
    bfb = np.asarray(inputs["bf"], np.float32)

    # BN fold into FC1
    scale = gamma / np.sqrt(var + BN_EPS)
    W1p = W1 * scale[None, :]
    b1p = b1f + W1 @ (beta - mean * scale)

    # feedback folds
    Wf2 = Wf @ W2  # [8, 64]
    bff = Wf @ b2 + bfb  # [8]
    W0r = W0x @ Wf2  # [4H, 64]
    b0r = W0x @ bff + b0  # [4H]

    # shared weight tiles
    w0x4 = np.zeros((128, 128), np.float32)
    b14 = np.zeros((128, 128), np.float32)
    for k, gname in enumerate(CHUNKS):
        rows = GATE_ROWS[gname]
        w0x4[32 * k, :] = b0[rows]
        w0x4[32 * k + 1 : 32 * k + 9, :] = W0x[rows].T
        if gname != "g":
            b14[32 * k, :] = b1[rows]

    def lhsT_of(W):
        out = np.zeros((H, 512), np.float32)
        for k, gname in enumerate(CHUNKS):
            out[:, 128 * k : 128 * (k + 1)] = W[GATE_ROWS[gname]].T
        return out

    w0r = np.zeros((FC_H + 1, 512), np.float32)
    for k, gname in enumerate(CHUNKS):
        rows = GATE_ROWS[gname]
        w0r[0:FC_H, 128 * k : 128 * (k + 1)] = W0r[rows].T
        w0r[FC_H, 128 * k : 128 * (k + 1)] = b0r[rows]

    shared = {
        "w0x4": w0x4.astype(bf),
        "b14": b14.astype(bf),
        "w0h": lhsT_of(W0h).astype(bf),
        "w1i": lhsT_of(W1i).astype(bf),
        "w1h": lhsT_of(W1h).astype(bf),
        "wfc1": W1p.T.astype(bf),
        "b1p": b1p.reshape(FC_H, 1).astype(np.float32),
        "wfc2": W2.T.astype(bf),
        "b2": b2.reshape(OUT, 1).astype(np.float32),
        "w0r": w0r.astype(bf),
        "b1g": b1[GATE_ROWS["g"]].reshape(H, 1).astype(np.float32),
    }

    in_maps = []
    bper = Bfull // N_CORES
    for ci in range(N_CORES):
        xc = x[ci * bper : (ci + 1) * bper]  # [B_loc, T, 8]
        xt = np.ascontiguousarray(xc.transpose(1, 2, 0))  # [T, 8, B_loc]
        x_enc = np.zeros((T, 128, bper), np.float32)
        for k in range(4):
            x_enc[:, 32 * k, :] = 1.0
            x_enc[:, 32 * k + 1 : 32 * k + 9, :] = xt
        m = dict(shared)
        m["x_enc"] = x_enc.astype(bf)
        in_maps.append(m)
    return in_maps


def kernel(**inputs) -> np.ndarray:
    x = np.asarray(inputs["x"])
    Bfull, T, _ = x.shape
    PRED = int(inputs["prediction_steps"])
    key = (T, PRED)
    if key not in _CACHE:
        _CACHE[key] = _build_program(T, PRED)
    nc = _CACHE[key]

    in_maps = _host_prep(inputs)
    trace = os.environ.get("KERNEL_TRACE", "0") == "1"
    if trace:
        try:
            from harness import install_ntff_hook

            install_ntff_hook()
        except Exception:
            trace = False
    res = run_bass_kernel_spmd(
        nc,
        in_maps,
        core_ids=list(range(N_CORES)),
        trace=trace,
        tmpdir=os.environ.get("KERNEL_TRACE_DIR") or None,
    )
    if trace and res.exec_time_ns is not None:
        print(f"HW exec time: {res.exec_time_ns} ns")

    bper = Bfull // N_CORES
    out = np.empty((Bfull, PRED, OUT), np.float32)
    for ci in range(N_CORES):
        y = res.results[ci]["y"]  # [PRED, OUT, B_loc]
        out[ci * bper : (ci + 1) * bper] = y.transpose(2, 0, 1)
    return out
